# revision 26
# baseline (speedup 1.0000x reference)
"""Trainium2 Bass kernel for nn_GCNNSingleKernel (gnn_message_passing).

Strategy:
- Data-parallel over batch B=16 across 8 NeuronCores (2 graphs per core).
- Graphs are SORTED by valid-node count nb and assigned to two per-core
  slots: slot0 gets the 8 largest graphs, slot1 the 8 smallest. Each slot
  is compiled with its own row-tile count T_r = ceil(max_nb/128) and
  column width C (nb rounded up), so all N^2 work (gram matmuls, exp,
  part3) and all N-wide streaming (DVE, conv) shrink to the valid region.
- adj_mask (B,N,N) is outer(node,node); the host extracts the diagonal so
  the 64MB mask never moves to the device.
- Per graph everything stays on-chip. The (N,N) Gaussian kernel matrix is
  produced by one matmul pass with two augmentation rows
  (T1=[emb;1;-sq/2], T2=[emb;-sq/2+BIGNEG*(node-1);1] so T1^T@T2 directly
  yields -d/2 plus a large negative offset on invalid-node columns) and
  one wide ACT Exp pass per 128-row tile. Invalid-node columns die inside
  the exp, so no (N,N) mask multiply is ever needed.
- Features live in a gapped 128-partition layout [conv 0:48 | zeros | res
  64:112 | zeros]; weights are host-padded to match. The residual matmul
  is merged into the first conv matmul (combined (128,128) stationary).
  deg rides as an extra column of the adj@emb^T matmul. Final
  InstanceNorm + fcl + sigmoid run on-device.
"""
import sys
import numpy as np

sys.path.insert(0, '/opt/trn_rl_repo')

import concourse.bass as bass  # noqa: E402
import concourse.tile as tile  # noqa: E402
from concourse import mybir  # noqa: E402

AF = mybir.ActivationFunctionType
ALU = mybir.AluOpType
FP = mybir.dt.float32
FR = mybir.dt.float32r
BF = mybir.dt.bfloat16

B, F0, FM, N, L, H = 16, 16, 96, 1024, 4, 48
NC = 8          # cores
BPC = B // NC   # graphs per core
NT = N // 128   # 8 n-tiles
EPS = 1e-5


def _width(nb):
    """Column width: nb rounded to 64; second 512-chunk kept >=256 wide
    (f32r matmuls below 256 free run at 1/4 rate)."""
    w = min(-(-int(nb) // 64) * 64, N)
    if 512 < w < 768:
        w = 768
    return max(w, 512)


def _chunks(C):
    return [(0, 512), (512, C - 512)] if C > 512 else [(0, C)]


def _split_multi_waits(nc, maxw=1):
    """Walrus (CoreV3) rejects >1 sync-wait on one instruction; spread extras
    onto same-engine NoOps inserted just before."""
    for f in nc.m.functions:
        for bb in f.blocks:
            newlist, changed = [], False
            for inst in bb.instructions:
                si = getattr(inst, 'sync_info', None)
                if si is not None and si.on_wait and len(si.on_wait) > maxw:
                    waits = list(si.on_wait)
                    head, tail = waits[:-maxw], waits[-maxw:]
                    for k in range(0, len(head), maxw):
                        nop = mybir.InstNoOp(
                            name=f'{inst.name}-w{k}', ins=[], outs=[])
                        nop.engine = inst.engine
                        nop.sync_info = mybir.SyncInfo(
                            on_wait=head[k:k + maxw], on_update=[])
                        newlist.append(nop)
                    si.on_wait = tail
                    inst.sync_info = si
                    changed = True
                newlist.append(inst)
            if changed:
                bb.instructions = newlist


def build_program(alphas, slots, reps=1, split_waits=True):
    """Per-core SPMD bass program. alphas: 4 floats baked as immediates.
    slots: ((T_r0, C0), (T_r1, C1)) row-tile count + column width per graph
    slot. reps>1 repeats the whole computation (timing variant).
    split_waits=False keeps multi-waits (CoreSim-friendly; walrus needs True).
    """
    nc = bass.Bass()

    emb_d = nc.dram_tensor('emb_in', [BPC, F0, N], FP, kind='ExternalInput')
    node_d = nc.dram_tensor('noderows', [BPC, 2, N], FP, kind='ExternalInput')
    ncol_d = nc.dram_tensor('node_colm', [BPC, 128, NT], FP, kind='ExternalInput')
    invnb_d = nc.dram_tensor('invnb', [BPC, 128, 1], FP, kind='ExternalInput')
    # first layer: k1+res merged stationary (16,128); k2 (16,64); k3 (16,64)
    wcr0_d = nc.dram_tensor('Wcr0', [F0, 128], FP, kind='ExternalInput')
    wc02_d = nc.dram_tensor('Wc02', [F0, 64], FP, kind='ExternalInput')
    wc03_d = nc.dram_tensor('Wc03', [F0, 128], FP, kind='ExternalInput')
    bc0_d = nc.dram_tensor('bc0', [64, 1], FP, kind='ExternalInput')
    br0_d = nc.dram_tensor('br0', [64, 1], FP, kind='ExternalInput')
    # layers 1..3: merged k1+res (128,128); k2 gapped (128,64); k3 (96,64)
    wcr_d = nc.dram_tensor('Wcr', [L - 1, 128, 128], FP, kind='ExternalInput')
    wc2_d = nc.dram_tensor('Wc2', [L - 1, 128, 64], FP, kind='ExternalInput')
    wc3_d = nc.dram_tensor('Wc3', [L - 1, FM, 128], FP, kind='ExternalInput')
    bc_d = nc.dram_tensor('bc', [L - 1, 64, 1], FP, kind='ExternalInput')
    br_d = nc.dram_tensor('br', [L - 1, 64, 1], FP, kind='ExternalInput')
    fclw_d = nc.dram_tensor('fclw', [128, 1], FP, kind='ExternalInput')
    id_d = nc.dram_tensor('ident', [128, 128], FP, kind='ExternalInput')
    scal_d = nc.dram_tensor('scal', [1, 4], FP, kind='ExternalInput')
    out_d = nc.dram_tensor('out', [BPC, 1], FP, kind='ExternalOutput')

    al = [float(a) for a in alphas]
    # n-side tensors (T1/T2/s2/EN/nbc/rA/rB) must span the full row-tile
    # range CL = max(C, TR*128); m-side streaming stays at C.
    CMAX = max(max(s[1], s[0] * 128) for s in slots)

    with tile.TileContext(nc) as tc:
        from contextlib import ExitStack
        with ExitStack() as ctx:
            const = ctx.enter_context(tc.tile_pool(name='const', bufs=1))
            p_raw = ctx.enter_context(tc.tile_pool(name='raw', bufs=2))
            p_nbc = ctx.enter_context(tc.tile_pool(name='nbc', bufs=2))
            p_T1 = ctx.enter_context(tc.tile_pool(name='T1', bufs=2))
            p_T2 = ctx.enter_context(tc.tile_pool(name='T2', bufs=2))
            p_s2 = ctx.enter_context(tc.tile_pool(name='s2', bufs=2))
            p_EN = ctx.enter_context(tc.tile_pool(name='EN', bufs=2))
            p_adj = ctx.enter_context(tc.tile_pool(name='adj', bufs=12))
            p_w = ctx.enter_context(tc.tile_pool(name='w', bufs=2))
            p_c2 = ctx.enter_context(tc.tile_pool(name='c2', bufs=2))
            p_c3 = ctx.enter_context(tc.tile_pool(name='c3', bufs=2))
            p_eo = ctx.enter_context(tc.tile_pool(name='eo', bufs=2))
            p_sm = ctx.enter_context(tc.tile_pool(name='sm', bufs=2))
            p_row = ctx.enter_context(tc.tile_pool(name='row', bufs=2))
            # PSUM: pg 2x(128,CMAX)=4 banks; acc 2x(128,512)=2; pmi 2x=2
            pg = ctx.enter_context(tc.tile_pool(name='ps_g', bufs=2, space='PSUM'))
            pacc = ctx.enter_context(tc.tile_pool(name='ps_a', bufs=2, space='PSUM'))
            pmi = ctx.enter_context(tc.tile_pool(name='ps_mi', bufs=2, space='PSUM'))

            # ---- constants ----
            id_sb = const.tile([128, 128], FP)
            nc.sync.dma_start(out=id_sb, in_=id_d[:, :])
            idb = const.tile([128, 128], BF)
            nc.vector.tensor_copy(idb, id_sb)
            ones_col = const.tile([128, 1], FP)
            nc.vector.memset(ones_col, 1.0)
            ones_row = const.tile([1, 128], FP)
            nc.vector.memset(ones_row, 1.0)
            ones_rowN = const.tile([1, N], FP)
            nc.vector.memset(ones_rowN, 1.0)
            wcr0_sb = const.tile([F0, 128], FP)
            nc.sync.dma_start(out=wcr0_sb, in_=wcr0_d[:, :])
            wc02_sb = const.tile([F0, 64], FP)
            nc.sync.dma_start(out=wc02_sb, in_=wc02_d[:, :])
            wc03_sb = const.tile([F0, 128], FP)
            nc.sync.dma_start(out=wc03_sb, in_=wc03_d[:, :])
            bc0_sb = const.tile([64, 1], FP)
            nc.sync.dma_start(out=bc0_sb, in_=bc0_d[:, :])
            br0_sb = const.tile([64, 1], FP)
            nc.sync.dma_start(out=br0_sb, in_=br0_d[:, :])
            wcr_sb = const.tile([128, L - 1, 128], FP)
            for ll in range(L - 1):
                nc.sync.dma_start(out=wcr_sb[:, ll, :], in_=wcr_d[ll, :, :])
            wc2_sb = const.tile([128, L - 1, 64], FP)
            for ll in range(L - 1):
                nc.sync.dma_start(out=wc2_sb[:, ll, :], in_=wc2_d[ll, :, :])
            wc3_sb = const.tile([FM, L - 1, 128], FP)
            for ll in range(L - 1):
                nc.sync.dma_start(out=wc3_sb[:, ll, :], in_=wc3_d[ll, :, :])
            bc_sb = const.tile([64, L - 1], FP)
            for ll in range(L - 1):
                nc.sync.dma_start(out=bc_sb[:, ll:ll + 1], in_=bc_d[ll, :, :])
            br_sb = const.tile([64, L - 1], FP)
            for ll in range(L - 1):
                nc.sync.dma_start(out=br_sb[:, ll:ll + 1], in_=br_d[ll, :, :])
            fclw_sb = const.tile([128, 1], FP)
            nc.sync.dma_start(out=fclw_sb, in_=fclw_d[:, :])
            scal_sb = const.tile([1, 4], FP)
            nc.sync.dma_start(out=scal_sb, in_=scal_d[:, :])
            # f32r-rounded weight copies (PE runs f32r matmuls single-pass)
            wcr0_b = const.tile([F0, 128], BF)
            nc.vector.tensor_copy(wcr0_b, wcr0_sb[:, :])
            wc02_b = const.tile([F0, 64], BF)
            nc.vector.tensor_copy(wc02_b, wc02_sb[:, :])
            wc03_b = const.tile([F0, 128], BF)
            nc.vector.tensor_copy(wc03_b, wc03_sb[:, :])
            wcr_b = const.tile([128, L - 1, 128], BF)
            nc.vector.tensor_copy(wcr_b, wcr_sb[:, :, :])
            wc2_b = const.tile([128, L - 1, 64], BF)
            nc.vector.tensor_copy(wc2_b, wc2_sb[:, :, :])
            wc3_b = const.tile([FM, L - 1, 128], BF)
            nc.vector.tensor_copy(wc3_b, wc3_sb[:, :, :])
            ones_row_r = const.tile([1, 128], FP)
            nc.vector.tensor_copy(ones_row_r[:, :].bitcast(FR), ones_row[:, :])
            zer16 = const.tile([128, 16], FP)
            nc.vector.memset(zer16, 0.0)
            ones_rowN_r = const.tile([1, N], FP)
            nc.vector.tensor_copy(ones_rowN_r[:, :].bitcast(FR),
                                  ones_rowN[:, :])
            ones_col_r = const.tile([128, 1], FP)
            nc.vector.tensor_copy(ones_col_r[:, :].bitcast(FR), ones_col[:, :])
            magic = const.tile([128, 1], mybir.dt.uint32)
            nc.vector.memset(magic, 0x5f3759df)

            def dve_rsqrt(out_ap, a_ap, np_):
                """out = 1/sqrt(a) via magic seed + 2 Newton iters (DVE only)."""
                U32 = mybir.dt.uint32
                z = p_sm.tile([128, 1], FP, tag='rsq_z')
                t = p_sm.tile([128, 1], FP, tag='rsq_t')
                zs = z[0:np_, :]
                ts = t[0:np_, :]
                nc.vector.tensor_scalar(zs.bitcast(U32), a_ap.bitcast(U32), 1,
                                        None, op0=ALU.logical_shift_right)
                nc.vector.scalar_tensor_tensor(
                    out=zs.bitcast(U32), in0=magic[0:np_, :], scalar=0,
                    in1=zs.bitcast(U32), op0=ALU.bypass, op1=ALU.subtract)
                for it in range(2):
                    nc.vector.tensor_tensor(ts, zs, zs, op=ALU.mult)
                    nc.vector.tensor_tensor(ts, ts, a_ap, op=ALU.mult)
                    nc.vector.tensor_scalar(ts, ts, -0.5, 1.5, op0=ALU.mult,
                                            op1=ALU.add)
                    dst = zs if it == 0 else out_ap
                    nc.vector.tensor_tensor(dst, zs, ts, op=ALU.mult)

            for rep in range(reps):
              G = [None] * BPC
              for b in range(BPC):
                TR, C = slots[b]
                CL = max(C, TR * 128)
                CHL = _chunks(CL)
                # ---- per-graph loads ----
                raw = p_raw.tile([F0, N], FP, tag='raw')
                nc.sync.dma_start(out=raw, in_=emb_d[b, :, :])
                raw_r = p_raw.tile([F0, N], BF, tag='rawr')
                nc.vector.tensor_copy(raw_r[:, :], raw[:, :])
                nrow = p_row.tile([1, N], FP, tag='nrow')
                nc.sync.dma_start(out=nrow, in_=node_d[b, 0:1, :])
                nrow_r = p_row.tile([1, N], FP, tag='nrowr')
                nc.vector.tensor_copy(nrow_r[:, :].bitcast(FR), nrow[:, :])
                nm1 = p_row.tile([1, N], FP, tag='nm1')
                nc.sync.dma_start(out=nm1, in_=node_d[b, 1:2, :])
                ncol = p_sm.tile([128, NT], FP, tag='ncol')
                nc.sync.dma_start(out=ncol, in_=ncol_d[b, :, :])
                invnb = p_sm.tile([128, 1], FP, tag='invnb')
                nc.sync.dma_start(out=invnb, in_=invnb_d[b, :, :])

                # node broadcast (128, CL) via K=1 matmuls
                nbc = p_nbc.tile([128, CMAX], FP, tag='nbc')
                for o, w in CHL:
                    pb = pmi.tile([128, 512], FP, tag='pmi')
                    nc.tensor.matmul(pb[:, 0:w],
                                     lhsT=ones_row_r[0:1, 0:128].bitcast(FR),
                                     rhs=nrow_r[:, o:o + w].bitcast(FR),
                                     start=True, stop=True)
                    nc.vector.tensor_copy(nbc[:, o:o + w], pb[:, 0:w])

                G[b] = dict(raw=raw, raw_r=raw_r, nrow=nrow,
                            nm1=nm1, ncol=ncol, invnb=invnb,
                            nbc=nbc, emb=raw, prev_c=None)
              for ll in range(L):
                for b in range(BPC):
                  TR, C = slots[b]
                  CL = max(C, TR * 128)
                  CH = _chunks(C)
                  CHL = _chunks(CL)
                  st = G[b]
                  raw = st['raw']; raw_r = st['raw_r']
                  nrow = st['nrow']; nm1 = st['nm1']
                  ncol = st['ncol']; invnb = st['invnb']
                  nbc = st['nbc']; emb = st['emb']
                  prev_c = st['prev_c']
                  if 1:
                    first = ll == 0
                    F = F0 if first else 128        # stored feature rows
                    KA = (F0 + 2) if first else 128  # gram contraction depth
                    a1 = F0 if first else 48         # aug row: ones/rB
                    a2 = F0 + 1 if first else 112    # aug row: rA/ones
                    alpha = al[ll]
                    c_l = 45.0 / alpha
                    do_norm = not first

                    T1 = p_T1.tile([KA, CMAX], FP, tag='T1')
                    s_col = p_sm.tile([128, 1], FP, tag='scol')
                    q_col = p_sm.tile([128, 1], FP, tag='qcol')
                    nc.vector.scalar_tensor_tensor(
                        out=T1[0:F, 0:CL].bitcast(FR),
                        in0=emb[:, 0:CL],
                        scalar=(prev_c[0:F, :] if do_norm else 1.0),
                        in1=nbc[0:F, 0:CL],
                        op0=(ALU.subtract if do_norm else ALU.mult),
                        op1=ALU.mult,
                        accum_out=s_col[0:F, :] if do_norm else None)
                    s2 = p_s2.tile([F, CMAX], FP, tag='s2')
                    nc.vector.scalar_tensor_tensor(
                        out=s2[:, 0:CL].bitcast(FR),
                        in0=T1[0:F, 0:CL], scalar=1.0, in1=T1[0:F, 0:CL],
                        op0=ALU.mult, op1=ALU.mult,
                        accum_out=q_col[0:F, :] if do_norm else None)
                    T2 = p_T2.tile([KA, CMAX], FP, tag='T2')
                    nc.vector.tensor_copy(T2[0:F, 0:CL].bitcast(FR),
                                          T1[0:F, 0:CL])

                    # aug rows: rA = -sq/2 ; rB = rA + c_l*(node-1)
                    rA = p_row.tile([1, N], FP, tag='rA')
                    rB = p_row.tile([1, N], FP, tag='rB')
                    for o, w in CHL:
                        sl = slice(o, o + w)
                        pr = pmi.tile([1, 512], FP, tag='pmi')
                        nc.tensor.matmul(
                            pr[:, 0:w],
                            lhsT=ones_col_r[0:F, 0:1].bitcast(FR),
                            rhs=s2[:, sl].bitcast(FR),
                            start=True, stop=True)
                        nc.vector.tensor_scalar(
                            rA[:, sl].bitcast(FR), pr[:, 0:w], -0.5,
                            None, op0=ALU.mult)
                        nc.vector.scalar_tensor_tensor(
                            out=rB[:, sl].bitcast(FR),
                            in0=nm1[:, sl], scalar=c_l,
                            in1=rA[:, sl], op0=ALU.mult, op1=ALU.add)
                    # DMA aug rows (DMA is partition-alignment-free);
                    # chunked so gram c=0 starts before c=1 rows land
                    nc.gpsimd.dma_start(out=T1[a1:a1 + 1, 0:CL].bitcast(FR),
                                        in_=ones_rowN_r[:, 0:CL])
                    nc.sync.dma_start(out=T2[a2:a2 + 1, 0:CL].bitcast(FR),
                                      in_=ones_rowN_r[:, 0:CL].bitcast(FR))
                    for o, w in CHL:
                        sl = slice(o, o + w)
                        nc.gpsimd.dma_start(
                            out=T1[a2:a2 + 1, sl].bitcast(FR), in_=rA[:, sl])
                        nc.sync.dma_start(
                            out=T2[a1:a1 + 1, sl].bitcast(FR),
                            in_=rB[:, sl].bitcast(FR))

                    if do_norm:
                        m = p_sm.tile([128, 1], FP, tag='m')
                        nc.vector.tensor_scalar_mul(m, s_col, invnb[:, 0:1])
                        qn = p_sm.tile([128, 1], FP, tag='qn')
                        nc.vector.tensor_scalar_mul(qn, q_col, invnb[:, 0:1])
                        mm = p_sm.tile([128, 1], FP, tag='mm')
                        nc.vector.tensor_mul(mm, m, m)
                        v = p_sm.tile([128, 1], FP, tag='v')
                        nc.vector.tensor_sub(v, qn, mm)
                        veps = p_sm.tile([128, 1], FP, tag='veps')
                        nc.vector.tensor_scalar(veps, v, EPS, None, op0=ALU.add)
                        inv = p_sm.tile([128, 1], FP, tag='inv')
                        dve_rsqrt(inv[:, :], veps[:, :], 128)
                        EN = p_EN.tile([128, CMAX], BF, tag='EN')
                        nc.vector.tensor_scalar(
                            EN[:, 0:CL],
                            T1[0:128, 0:CL], m, inv,
                            op0=ALU.subtract, op1=ALU.mult)
                        cat1 = EN
                        wsrc = EN
                    else:
                        cat1 = raw_r
                        T1b = p_EN.tile([F0, CMAX], BF, tag='ENb')
                        nc.vector.tensor_copy(T1b[:, 0:CL], T1[0:F0, 0:CL])
                        wsrc = T1b  # rows 0:16 = masked raw

                    # gram + wide exp -> adj tiles (adj[t] holds rows t*128..)
                    adj_t = []
                    for t in range(TR):
                        at = p_adj.tile([128, CMAX], BF, tag='adj')
                        adj_t.append(at)
                        pgt = pg.tile([128, CMAX], FP, tag='pg')
                        for o, w in CH:
                            nc.tensor.matmul(
                                pgt[:, o:o + w],
                                lhsT=T2[:, t * 128:(t + 1) * 128].bitcast(FR),
                                rhs=T1[:, o:o + w].bitcast(FR),
                                start=True, stop=True)
                        nc.scalar.activation(
                            at[:, 0:C], pgt[:, 0:C],
                            AF.Exp, bias=0.0, scale=2.0 * alpha)

                    # w blocks (128, WK): transposed masked features + node col
                    WK = 33 if first else 97
                    w_sb = p_w.tile([128, NT * WK], BF, tag='w')
                    if first:
                        for t in range(TR):
                            nc.vector.memset(
                                w_sb[:, t * 33 + 16:t * 33 + 32], 0.0)
                    for t in range(TR):
                        tsl = slice(t * 128, (t + 1) * 128)
                        if first:
                            pt = pmi.tile([128, 512], BF, tag='pmi')
                            nc.tensor.transpose(pt[:, 0:F0], wsrc[0:F0, tsl],
                                                idb[0:F0, 0:F0])
                            nc.vector.tensor_copy(
                                w_sb[:, t * WK:t * WK + F0], pt[:, 0:F0])
                        else:
                            pt = pmi.tile([128, 512], BF, tag='pmi')
                            nc.tensor.transpose(pt[:, 0:H], wsrc[0:H, tsl],
                                                idb[0:H, 0:H])
                            nc.vector.tensor_scalar_mul(
                                w_sb[:, t * WK:t * WK + H],
                                pt[:, 0:H], ncol[:, t:t + 1])
                            pt2 = pmi.tile([128, 512], BF, tag='pmi')
                            nc.tensor.transpose(pt2[:, 0:H],
                                                wsrc[64:112, tsl],
                                                idb[64:112, 0:H])
                            nc.vector.tensor_scalar_mul(
                                w_sb[:, t * WK + H:t * WK + FM],
                                pt2[:, 0:H], ncol[:, t:t + 1])
                        nc.vector.tensor_copy(
                            w_sb[:, t * WK + WK - 1:t * WK + WK],
                            ncol[:, t:t + 1])

                    # part3 (w^T @ adj) -> cat3 + deg row; t-outer so each
                    # stationary w block feeds both column chunks
                    NF3 = F0 if first else FM
                    cat3 = p_c3.tile([NF3, CMAX], BF, tag='c3')
                    degrow = p_row.tile([1, N], FP, tag='degrow')
                    for ci, (o, w) in enumerate(CH):
                        pp_ = pacc.tile([128, 512], FP, tag='acc')
                        for t in range(TR):
                            nc.tensor.matmul(
                                pp_[0:WK, 0:w],
                                lhsT=w_sb[:, t * WK:(t + 1) * WK],
                                rhs=adj_t[t][:, o:o + w],
                                start=(t == 0), stop=(t == TR - 1))
                        nc.vector.tensor_copy(
                            cat3[:, o:o + w],
                            pp_[0:NF3, 0:w])
                        nc.vector.tensor_copy(
                            degrow[:, o:o + w].bitcast(FR),
                            pp_[WK - 1:WK, 0:w])

                    # cat2 = cat1 * deg (broadcast deg via K=1 matmul)
                    FC = F0 if first else 128
                    cat2 = p_c2.tile([FC, CMAX], BF, tag='c2')
                    for o, w in CH:
                        sl = slice(o, o + w)
                        pd = pmi.tile([128, 512], FP, tag='pmi')
                        nc.tensor.matmul(
                            pd[0:FC, 0:w],
                            lhsT=ones_row_r[0:1, 0:FC].bitcast(FR),
                            rhs=degrow[:, sl].bitcast(FR),
                            start=True, stop=True)
                        nc.vector.tensor_tensor(
                            cat2[:, sl],
                            cat1[:, sl], pd[0:FC, 0:w], op=ALU.mult)

                    # conv+res merged -> emb_out (gapped layout); weights are
                    # zero-padded so gap rows come out exactly 0.
                    # k=1 stationary [Wc1|Wr] (K,128) writes all 128 rows;
                    # k=2 (cat2) and k=3 (cat3) accumulate rows 0:64.
                    emb_out = p_eo.tile([128, CMAX], FP, tag='eo')
                    cacc = p_sm.tile([128, 2], FP, tag='cacc')
                    w1 = wcr0_b if first else wcr_b[:, ll - 1, :]
                    w2 = wc02_b if first else wc2_b[:, ll - 1, :]
                    w3 = wc03_b if first else wc3_b[:, ll - 1, :]
                    # per chunk, one contiguous group: merged [Wc1|Wr]
                    # opens on all 128 rows (start), k2 accumulates rows
                    # 0:64, k3 (stationary zero-padded to 128 output rows)
                    # closes — rows 64:128 accumulate +0, so the stop covers
                    # every partition. HW-validated pattern (mb3).
                    for ci, (o, w) in enumerate(CH):
                        sl = slice(o, o + w)
                        pc_ = pacc.tile([128, 512], FP, tag='acc')
                        nc.tensor.matmul(
                            pc_[:, 0:w], lhsT=w1,
                            rhs=cat1[:, sl],
                            start=True, stop=False)
                        nc.tensor.matmul(
                            pc_[0:64, 0:w], lhsT=w2,
                            rhs=cat2[:, sl],
                            start=False, stop=False)
                        nc.tensor.matmul(
                            pc_[:, 0:w], lhsT=w3,
                            rhs=cat3[:, sl],
                            start=False, stop=True)
                        nc.scalar.activation(
                            emb_out[0:64, sl], pc_[0:64, 0:w], AF.Relu,
                            bias=(bc0_sb[:, 0:1] if first
                                  else bc_sb[:, ll - 1:ll]), scale=1.0,
                            accum_out=cacc[0:64, ci:ci + 1])
                        nc.scalar.activation(
                            emb_out[64:128, sl], pc_[64:128, 0:w],
                            AF.Identity,
                            bias=(br0_sb[:, 0:1] if first
                                  else br_sb[:, ll - 1:ll]), scale=1.0,
                            accum_out=cacc[64:128, ci:ci + 1])
                    if CL > C:
                        # next layer's T1 reads cols 0:CL; zero the pad
                        nc.vector.memset(emb_out[:, C:CL], 0.0)
                    emb = emb_out
                    ccol = p_sm.tile([128, 1], FP, tag='ccol')
                    if len(CH) > 1:
                        nc.vector.tensor_tensor(ccol, cacc[:, 0:1],
                                                cacc[:, 1:2], op=ALU.add)
                        nc.vector.tensor_scalar(ccol, ccol, 1.0 / N, None,
                                                op0=ALU.mult)
                    else:
                        nc.vector.tensor_scalar(ccol, cacc[:, 0:1], 1.0 / N,
                                                None, op0=ALU.mult)
                    prev_c = ccol
                  st['emb'] = emb
                  st['prev_c'] = prev_c
              for b in range(BPC):
                TR, C = slots[b]
                st = G[b]
                nbc = st['nbc']; emb = st['emb']
                invnb = st['invnb']
                # ---- epilogue ----
                fm = p_s2.tile([128, CMAX], FP, tag='s2')
                pooled = p_sm.tile([128, 1], FP, tag='pooled')
                nc.vector.scalar_tensor_tensor(
                    out=fm[:, 0:C], in0=emb[:, 0:C], scalar=1.0,
                    in1=nbc[:, 0:C],
                    op0=ALU.mult, op1=ALU.mult, accum_out=pooled)
                p2 = p_sm.tile([128, 1], FP, tag='p2')
                nc.vector.tensor_mul(p2, pooled, pooled)
                pair = p_sm.tile([128, 2], FP, tag='pair')
                nc.vector.tensor_copy(pair[:, 0:1], pooled)
                nc.vector.tensor_copy(pair[:, 1:2], p2)
                psA = pmi.tile([1, 512], FP, tag='pmi')
                nc.tensor.matmul(psA[:, 0:2], lhsT=ones_col[:, 0:1],
                                 rhs=pair, start=True, stop=True)
                psB = pmi.tile([1, 512], FP, tag='pmi')
                nc.tensor.matmul(psB[:, 0:1], lhsT=fclw_sb,
                                 rhs=pooled, start=True, stop=True)
                sc = p_sm.tile([1, 8], FP, tag='sc')
                nc.vector.tensor_scalar(sc[:, 0:1], psA[:, 0:1], 1.0 / FM,
                                        None, op0=ALU.mult)   # mbar
                nc.vector.tensor_scalar(sc[:, 1:2], psA[:, 1:2], 1.0 / FM,
                                        None, op0=ALU.mult)   # qbar
                nc.vector.tensor_mul(sc[:, 2:3], sc[:, 0:1], sc[:, 0:1])
                nc.vector.tensor_sub(sc[:, 3:4], sc[:, 1:2], sc[:, 2:3])  # v
                nc.vector.tensor_scalar(sc[:, 4:5], sc[:, 3:4], EPS, None,
                                        op0=ALU.add)
                dve_rsqrt(sc[:, 5:6], sc[:, 4:5], 1)  # rv
                nc.vector.tensor_scalar_mul(sc[:, 6:7], sc[:, 0:1],
                                            scal_sb[:, 0:1])
                nc.vector.tensor_sub(sc[:, 7:8], psB[:, 0:1], sc[:, 6:7])
                sc2 = p_sm.tile([1, 2], FP, tag='sc2')
                nc.vector.tensor_mul(sc2[:, 0:1], sc[:, 7:8], sc[:, 5:6])
                # sigmoid(z+fb) = 1/(1+exp(-z-fb)); scal[0,2] = -fb
                nc.scalar.activation(sc2[:, 1:2], sc2[:, 0:1], AF.Exp,
                                     bias=scal_sb[:, 2:3], scale=-1.0)
                outsb = p_sm.tile([1, 1], FP, tag='outsb')
                nc.vector.tensor_scalar(outsb, sc2[:, 1:2], 1.0, None,
                                        op0=ALU.add)
                nc.vector.reciprocal(outsb, outsb)
                nc.sync.dma_start(out=out_d[b:b + 1, :], in_=outsb)

    if split_waits:
        _split_multi_waits(nc)
    return nc


_CACHE = {}
_RUN_CACHE = {}
_LAST_INMAPS = None
_LAST_SLOTS = None


def _get_program(alphas, slots):
    key = (tuple(float(a) for a in alphas), slots)
    if key not in _CACHE:
        _CACHE[key] = build_program(alphas, slots)
    return _CACHE[key]


def _gap(a96, axis=0):
    """Gapped-128 feature layout: [0:48]=f[0:48], [64:112]=f[48:96]."""
    shp = list(a96.shape)
    shp[axis] = 128
    out = np.zeros(shp, a96.dtype)
    idx0 = [slice(None)] * a96.ndim
    idx1 = [slice(None)] * a96.ndim
    src0 = [slice(None)] * a96.ndim
    src1 = [slice(None)] * a96.ndim
    idx0[axis] = slice(0, 48); src0[axis] = slice(0, 48)
    idx1[axis] = slice(64, 112); src1[axis] = slice(48, 96)
    out[tuple(idx0)] = a96[tuple(src0)]
    out[tuple(idx1)] = a96[tuple(src1)]
    return out


def kernel(**inputs):
    ins = {k: np.asarray(v) for k, v in inputs.items()}
    emb_in = ins['emb_in'].astype(np.float32)
    adj_mask = ins['adj_mask']
    nb = ins['batch_nb_nodes'].astype(np.float64)
    alphas = ins['alphas'].astype(np.float32)

    node = np.ascontiguousarray(
        np.einsum('bii->bi', adj_mask)).astype(np.float32)       # (B,N)
    noderows = np.ascontiguousarray(
        np.stack([node, node - 1.0], axis=1))                    # (B,2,N)
    node_colm = np.ascontiguousarray(
        node.reshape(B, NT, 128).transpose(0, 2, 1))             # (B,128,NT)
    invnb = np.ascontiguousarray(np.repeat(
        (1.0 / nb).astype(np.float32)[:, None], 128, axis=1)[:, :, None])

    def pad64(a):  # pad last axis H->64 with zeros
        shp = list(a.shape); shp[-1] = 64 - a.shape[-1]
        return np.ascontiguousarray(
            np.concatenate([a, np.zeros(shp, a.dtype)], axis=-1))

    # first layer: Wc0 (48,48) -> T (48,48) = 3 chunks (16,48)
    Wc0T = np.ascontiguousarray(ins['Wc0'].astype(np.float32).T)  # (48,48)
    wc0c = Wc0T.reshape(3, F0, H)
    Wr0T = np.ascontiguousarray(ins['Wr0'].astype(np.float32).T)  # (16,48)
    wcr0 = np.zeros((F0, 128), np.float32)
    wcr0[:, 0:H] = wc0c[0]
    wcr0[:, 64:112] = Wr0T
    wc02 = pad64(np.ascontiguousarray(wc0c[1]))
    wc03 = np.zeros((F0, 128), np.float32)
    wc03[:, 0:H] = wc0c[2]
    bc0 = pad64(ins['bc0'].astype(np.float32).reshape(1, H)).reshape(64, 1)
    br0 = pad64(ins['br0'].astype(np.float32).reshape(1, H)).reshape(64, 1)
    # layer >=1: Wc[l] (48,288) -> T (288,48) -> 3 chunks (96,48);
    # chunk1 merged with Wr into (128,128); chunk2 gapped; chunk3 compact.
    wcr = np.zeros((L - 1, 128, 128), np.float32)
    wc2 = np.zeros((L - 1, 128, 64), np.float32)
    wc3 = np.zeros((L - 1, FM, 128), np.float32)
    for i in range(L - 1):
        WcT = ins['Wc'][i].astype(np.float32).T        # (288,48)
        wcr[i, :, 0:H] = _gap(WcT[0:96])
        wcr[i, :, 64:112] = _gap(ins['Wr'][i].astype(np.float32).T)
        wc2[i, :, 0:H] = _gap(WcT[96:192])
        wc3[i, :, 0:H] = WcT[192:288]
    bc = pad64(ins['bc'].astype(np.float32)).reshape(L - 1, 64, 1)
    br = pad64(ins['br'].astype(np.float32)).reshape(L - 1, 64, 1)
    fclw = _gap(ins['fcl_w'].astype(np.float32).reshape(FM, 1))
    ident = np.zeros((128, 128), np.float32)
    ident[:64, :64] = np.eye(64)
    ident[64:112, 0:48] = np.eye(48)
    fb = float(ins['fcl_b'].reshape(-1)[0])
    scal = np.array([[float(ins['fcl_w'].sum()), fb, -fb, 0.0]], np.float32)

    # sort graphs by nb desc; slot0 = 8 largest, slot1 = 8 smallest
    nbi = ins['batch_nb_nodes'].astype(np.int64)
    order = np.argsort(-nbi, kind='stable')
    slots = tuple(
        (int(-(-int(nbi[order[s * NC]]) // 128)), _width(nbi[order[s * NC]]))
        for s in range(BPC))

    in_maps = []
    for k in range(NC):
        g = [int(order[k]), int(order[NC + k])]
        in_maps.append({
            'emb_in': np.ascontiguousarray(emb_in[g]),
            'noderows': np.ascontiguousarray(noderows[g]),
            'node_colm': np.ascontiguousarray(node_colm[g]),
            'invnb': np.ascontiguousarray(invnb[g]),
            'Wcr0': wcr0, 'Wc02': wc02, 'Wc03': wc03,
            'bc0': bc0, 'br0': br0,
            'Wcr': wcr, 'Wc2': wc2, 'Wc3': wc3, 'bc': bc, 'br': br,
            'fclw': fclw, 'ident': ident, 'scal': scal,
        })

    global _LAST_INMAPS, _LAST_SLOTS
    _LAST_INMAPS = in_maps
    _LAST_SLOTS = slots
    runner = _get_runner(tuple(float(a) for a in alphas), slots)
    outs = runner(in_maps)
    out = np.zeros((B,), np.float32)
    for k in range(NC):
        out[order[k]] = outs[k][0, 0]
        out[order[NC + k]] = outs[k][1, 0]
    return out.astype(np.float32)


def _get_runner(key, slots, reps=1):
    """Persistent jitted SPMD executor (avoids per-call jax retracing)."""
    ck = (key, slots, reps)
    if ck in _RUN_CACHE:
        return _RUN_CACHE[ck]
    import jax
    from jax.experimental.shard_map import shard_map
    from jax.sharding import Mesh, PartitionSpec
    from concourse import bass2jax, mybir as _mb

    pk = (key, slots) if reps == 1 else ck
    if pk not in _CACHE:
        _CACHE[pk] = build_program(list(key), slots, reps)
    nc_prog = _CACHE[pk]
    bass2jax.install_neuronx_cc_hook()

    pname = (nc_prog.partition_id_tensor.name
             if nc_prog.partition_id_tensor else None)
    in_names, out_names, out_avals, zero_outs = [], [], [], []
    for alloc in nc_prog.m.functions[0].allocations:
        if not isinstance(alloc, _mb.MemoryLocationSet):
            continue
        name = alloc.memorylocations[0].name
        if alloc.kind == 'ExternalInput':
            if name != pname:
                in_names.append(name)
        elif alloc.kind == 'ExternalOutput':
            out_names.append(name)
            shape = tuple(alloc.tensor_shape)
            dtype = _mb.dt.np(alloc.dtype)
            out_avals.append(jax.core.ShapedArray(shape, dtype))
            zero_outs.append(np.zeros(shape, dtype))
    n_params = len(in_names)
    all_names = in_names + out_names + ([pname] if pname else [])

    def _body(*args):
        operands = list(args)
        if pname:
            operands.append(bass2jax.partition_id_tensor())
        outs = bass2jax._bass_exec_p.bind(
            *operands, out_avals=tuple(out_avals), in_names=tuple(all_names),
            out_names=tuple(out_names), lowering_input_output_aliases=(),
            sim_require_finite=True, sim_require_nnan=True, nc=nc_prog)
        return tuple(outs)

    devices = jax.devices()[:NC]
    mesh = Mesh(np.asarray(devices), ('core',))
    n_outs = len(out_names)
    sharded = jax.jit(
        shard_map(_body, mesh=mesh,
                  in_specs=(PartitionSpec('core'),) * (n_params + n_outs),
                  out_specs=(PartitionSpec('core'),) * n_outs,
                  check_rep=False),
        keep_unused=True)

    def run(in_maps):
        concat_in = [np.concatenate([np.asarray(m[nm]) for m in in_maps],
                                    axis=0) for nm in in_names]
        concat_zero = [np.zeros((NC * z.shape[0], *z.shape[1:]), z.dtype)
                       for z in zero_outs]
        out_arrs = sharded(*concat_in, *concat_zero)
        o = np.asarray(out_arrs[0]).reshape(NC, *out_avals[0].shape)
        return [o[c] for c in range(NC)]

    _RUN_CACHE[ck] = run
    return run


if __name__ == '__main__':
    sys.path.insert(0, '/root/problem')
    import jax
    import reference as R
    cpu = jax.devices('cpu')[0]
    with jax.default_device(cpu):
        inp = {k: np.asarray(v) for k, v in R.setup_inputs().items()}
        exp = np.asarray(R.reference(**R.setup_inputs()))
    got = kernel(**inp)
    rel = np.abs(got - exp) / (np.abs(exp) + 1e-9)
    print('expected:', exp[:8])
    print('got     :', got[:8])
    print('max rel err:', rel.max())


# revision 27
# speedup vs baseline: 1.0201x; 1.0201x over previous
"""Trainium2 Bass kernel for nn_GCNNSingleKernel (gnn_message_passing).

Strategy:
- Data-parallel over batch B=16 across 8 NeuronCores (2 graphs per core).
- Graphs are SORTED by valid-node count nb and assigned to two per-core
  slots: slot0 gets the 8 largest graphs, slot1 the 8 smallest. Each slot
  is compiled with its own row-tile count T_r = ceil(max_nb/128) and
  column width C (nb rounded up), so all N^2 work (gram matmuls, exp,
  part3) and all N-wide streaming (DVE, conv) shrink to the valid region.
- adj_mask (B,N,N) is outer(node,node); the host extracts the diagonal so
  the 64MB mask never moves to the device.
- Per graph everything stays on-chip. The (N,N) Gaussian kernel matrix is
  produced by one matmul pass with two augmentation rows
  (T1=[emb;1;-sq/2], T2=[emb;-sq/2+BIGNEG*(node-1);1] so T1^T@T2 directly
  yields -d/2 plus a large negative offset on invalid-node columns) and
  one wide ACT Exp pass per 128-row tile. Invalid-node columns die inside
  the exp, so no (N,N) mask multiply is ever needed.
- Features live in a gapped 128-partition layout [conv 0:48 | zeros | res
  64:112 | zeros]; weights are host-padded to match. The residual matmul
  is merged into the first conv matmul (combined (128,128) stationary).
  deg rides as an extra column of the adj@emb^T matmul. Final
  InstanceNorm + fcl + sigmoid run on-device.
"""
import sys
import numpy as np

sys.path.insert(0, '/opt/trn_rl_repo')

import concourse.bass as bass  # noqa: E402
import concourse.tile as tile  # noqa: E402
from concourse import mybir  # noqa: E402

AF = mybir.ActivationFunctionType
ALU = mybir.AluOpType
FP = mybir.dt.float32
FR = mybir.dt.float32r
BF = mybir.dt.bfloat16

B, F0, FM, N, L, H = 16, 16, 96, 1024, 4, 48
NC = 8          # cores
BPC = B // NC   # graphs per core
NT = N // 128   # 8 n-tiles
EPS = 1e-5


def _width(nb):
    """Column width: nb rounded to 64; second 512-chunk kept >=256 wide
    (f32r matmuls below 256 free run at 1/4 rate)."""
    w = min(-(-int(nb) // 64) * 64, N)
    if 512 < w < 768:
        w = 768
    return max(w, 512)


def _chunks(C):
    return [(0, 512), (512, C - 512)] if C > 512 else [(0, C)]


def _split_multi_waits(nc, maxw=1):
    """Walrus (CoreV3) rejects >1 sync-wait on one instruction; spread extras
    onto same-engine NoOps inserted just before."""
    for f in nc.m.functions:
        for bb in f.blocks:
            newlist, changed = [], False
            for inst in bb.instructions:
                si = getattr(inst, 'sync_info', None)
                if si is not None and si.on_wait and len(si.on_wait) > maxw:
                    waits = list(si.on_wait)
                    head, tail = waits[:-maxw], waits[-maxw:]
                    for k in range(0, len(head), maxw):
                        nop = mybir.InstNoOp(
                            name=f'{inst.name}-w{k}', ins=[], outs=[])
                        nop.engine = inst.engine
                        nop.sync_info = mybir.SyncInfo(
                            on_wait=head[k:k + maxw], on_update=[])
                        newlist.append(nop)
                    si.on_wait = tail
                    inst.sync_info = si
                    changed = True
                newlist.append(inst)
            if changed:
                bb.instructions = newlist


def build_program(alphas, slots, reps=1, split_waits=True):
    """Per-core SPMD bass program. alphas: 4 floats baked as immediates.
    slots: ((T_r0, C0), (T_r1, C1)) row-tile count + column width per graph
    slot. reps>1 repeats the whole computation (timing variant).
    split_waits=False keeps multi-waits (CoreSim-friendly; walrus needs True).
    """
    nc = bass.Bass()

    emb_d = nc.dram_tensor('emb_in', [BPC, F0, N], FP, kind='ExternalInput')
    node_d = nc.dram_tensor('noderows', [BPC, 2, N], FP, kind='ExternalInput')
    ncol_d = nc.dram_tensor('node_colm', [BPC, 128, NT], FP, kind='ExternalInput')
    invnb_d = nc.dram_tensor('invnb', [BPC, 128, 1], FP, kind='ExternalInput')
    # first layer: k1+res merged stationary (16,128); k2 (16,64); k3 (16,64)
    wcr0_d = nc.dram_tensor('Wcr0', [F0, 128], FP, kind='ExternalInput')
    wc02_d = nc.dram_tensor('Wc02', [F0, 64], FP, kind='ExternalInput')
    wc03_d = nc.dram_tensor('Wc03', [F0, 128], FP, kind='ExternalInput')
    bc0_d = nc.dram_tensor('bc0', [64, 1], FP, kind='ExternalInput')
    br0_d = nc.dram_tensor('br0', [64, 1], FP, kind='ExternalInput')
    # layers 1..3: merged k1+res (128,128); k2 gapped (128,64); k3 (96,64)
    wcr_d = nc.dram_tensor('Wcr', [L - 1, 128, 128], FP, kind='ExternalInput')
    wc2_d = nc.dram_tensor('Wc2', [L - 1, 128, 64], FP, kind='ExternalInput')
    wc3_d = nc.dram_tensor('Wc3', [L - 1, FM, 128], FP, kind='ExternalInput')
    bc_d = nc.dram_tensor('bc', [L - 1, 64, 1], FP, kind='ExternalInput')
    br_d = nc.dram_tensor('br', [L - 1, 64, 1], FP, kind='ExternalInput')
    fclw_d = nc.dram_tensor('fclw', [128, 1], FP, kind='ExternalInput')
    id_d = nc.dram_tensor('ident', [128, 128], FP, kind='ExternalInput')
    scal_d = nc.dram_tensor('scal', [1, 4], FP, kind='ExternalInput')
    out_d = nc.dram_tensor('out', [BPC, 1], FP, kind='ExternalOutput')

    al = [float(a) for a in alphas]
    # n-side tensors (T1/T2/s2/EN/nbc/rA/rB) must span the full row-tile
    # range CL = max(C, TR*128); m-side streaming stays at C.
    CMAX = max(max(s[1], s[0] * 128) for s in slots)

    with tile.TileContext(nc) as tc:
        from contextlib import ExitStack
        with ExitStack() as ctx:
            const = ctx.enter_context(tc.tile_pool(name='const', bufs=1))
            p_raw = ctx.enter_context(tc.tile_pool(name='raw', bufs=2))
            p_nbc = ctx.enter_context(tc.tile_pool(name='nbc', bufs=2))
            p_T1 = ctx.enter_context(tc.tile_pool(name='T1', bufs=2))
            p_T2 = ctx.enter_context(tc.tile_pool(name='T2', bufs=2))
            p_s2 = ctx.enter_context(tc.tile_pool(name='s2', bufs=2))
            p_EN = ctx.enter_context(tc.tile_pool(name='EN', bufs=2))
            p_adj = ctx.enter_context(tc.tile_pool(name='adj', bufs=12))
            p_w = ctx.enter_context(tc.tile_pool(name='w', bufs=2))
            p_c2 = ctx.enter_context(tc.tile_pool(name='c2', bufs=2))
            p_c3 = ctx.enter_context(tc.tile_pool(name='c3', bufs=2))
            p_eo = ctx.enter_context(tc.tile_pool(name='eo', bufs=2))
            p_sm = ctx.enter_context(tc.tile_pool(name='sm', bufs=2))
            p_row = ctx.enter_context(tc.tile_pool(name='row', bufs=2))
            # PSUM: pg 2x(128,CMAX)=4 banks; acc 2x(128,512)=2; pmi 2x=2
            pg = ctx.enter_context(tc.tile_pool(name='ps_g', bufs=2, space='PSUM'))
            pacc = ctx.enter_context(tc.tile_pool(name='ps_a', bufs=2, space='PSUM'))
            pmi = ctx.enter_context(tc.tile_pool(name='ps_mi', bufs=2, space='PSUM'))

            # ---- constants ----
            id_sb = const.tile([128, 128], FP)
            nc.sync.dma_start(out=id_sb, in_=id_d[:, :])
            idb = const.tile([128, 128], BF)
            nc.vector.tensor_copy(idb, id_sb)
            ones_col = const.tile([128, 1], FP)
            nc.vector.memset(ones_col, 1.0)
            ones_row = const.tile([1, 128], FP)
            nc.vector.memset(ones_row, 1.0)
            ones_rowN = const.tile([1, N], FP)
            nc.vector.memset(ones_rowN, 1.0)
            wcr0_sb = const.tile([F0, 128], FP)
            nc.sync.dma_start(out=wcr0_sb, in_=wcr0_d[:, :])
            wc02_sb = const.tile([F0, 64], FP)
            nc.sync.dma_start(out=wc02_sb, in_=wc02_d[:, :])
            wc03_sb = const.tile([F0, 128], FP)
            nc.sync.dma_start(out=wc03_sb, in_=wc03_d[:, :])
            bc0_sb = const.tile([64, 1], FP)
            nc.sync.dma_start(out=bc0_sb, in_=bc0_d[:, :])
            br0_sb = const.tile([64, 1], FP)
            nc.sync.dma_start(out=br0_sb, in_=br0_d[:, :])
            wcr_sb = const.tile([128, L - 1, 128], FP)
            for ll in range(L - 1):
                nc.sync.dma_start(out=wcr_sb[:, ll, :], in_=wcr_d[ll, :, :])
            wc2_sb = const.tile([128, L - 1, 64], FP)
            for ll in range(L - 1):
                nc.sync.dma_start(out=wc2_sb[:, ll, :], in_=wc2_d[ll, :, :])
            wc3_sb = const.tile([FM, L - 1, 128], FP)
            for ll in range(L - 1):
                nc.sync.dma_start(out=wc3_sb[:, ll, :], in_=wc3_d[ll, :, :])
            bc_sb = const.tile([64, L - 1], FP)
            for ll in range(L - 1):
                nc.sync.dma_start(out=bc_sb[:, ll:ll + 1], in_=bc_d[ll, :, :])
            br_sb = const.tile([64, L - 1], FP)
            for ll in range(L - 1):
                nc.sync.dma_start(out=br_sb[:, ll:ll + 1], in_=br_d[ll, :, :])
            fclw_sb = const.tile([128, 1], FP)
            nc.sync.dma_start(out=fclw_sb, in_=fclw_d[:, :])
            scal_sb = const.tile([1, 4], FP)
            nc.sync.dma_start(out=scal_sb, in_=scal_d[:, :])
            # f32r-rounded weight copies (PE runs f32r matmuls single-pass)
            wcr0_r = const.tile([F0, 128], FP)
            nc.vector.tensor_copy(wcr0_r[:, :].bitcast(FR), wcr0_sb[:, :])
            wc02_r = const.tile([F0, 64], FP)
            nc.vector.tensor_copy(wc02_r[:, :].bitcast(FR), wc02_sb[:, :])
            wc03_r = const.tile([F0, 128], FP)
            nc.vector.tensor_copy(wc03_r[:, :].bitcast(FR), wc03_sb[:, :])
            wcr_r = const.tile([128, L - 1, 128], FP)
            nc.vector.tensor_copy(wcr_r[:, :, :].bitcast(FR),
                                  wcr_sb[:, :, :])
            wc2_r = const.tile([128, L - 1, 64], FP)
            nc.vector.tensor_copy(wc2_r[:, :, :].bitcast(FR), wc2_sb[:, :, :])
            wc3_r = const.tile([FM, L - 1, 128], FP)
            nc.vector.tensor_copy(wc3_r[:, :, :].bitcast(FR), wc3_sb[:, :, :])
            ones_row_r = const.tile([1, 128], FP)
            nc.vector.tensor_copy(ones_row_r[:, :].bitcast(FR), ones_row[:, :])
            zer16 = const.tile([128, 16], FP)
            nc.vector.memset(zer16, 0.0)
            ones_rowN_r = const.tile([1, N], FP)
            nc.vector.tensor_copy(ones_rowN_r[:, :].bitcast(FR),
                                  ones_rowN[:, :])
            ones_col_r = const.tile([128, 1], FP)
            nc.vector.tensor_copy(ones_col_r[:, :].bitcast(FR), ones_col[:, :])
            magic = const.tile([128, 1], mybir.dt.uint32)
            nc.vector.memset(magic, 0x5f3759df)

            def dve_rsqrt(out_ap, a_ap, np_):
                """out = 1/sqrt(a) via magic seed + 2 Newton iters (DVE only)."""
                U32 = mybir.dt.uint32
                z = p_sm.tile([128, 1], FP, tag='rsq_z')
                t = p_sm.tile([128, 1], FP, tag='rsq_t')
                zs = z[0:np_, :]
                ts = t[0:np_, :]
                nc.vector.tensor_scalar(zs.bitcast(U32), a_ap.bitcast(U32), 1,
                                        None, op0=ALU.logical_shift_right)
                nc.vector.scalar_tensor_tensor(
                    out=zs.bitcast(U32), in0=magic[0:np_, :], scalar=0,
                    in1=zs.bitcast(U32), op0=ALU.bypass, op1=ALU.subtract)
                for it in range(2):
                    nc.vector.tensor_tensor(ts, zs, zs, op=ALU.mult)
                    nc.vector.tensor_tensor(ts, ts, a_ap, op=ALU.mult)
                    nc.vector.tensor_scalar(ts, ts, -0.5, 1.5, op0=ALU.mult,
                                            op1=ALU.add)
                    dst = zs if it == 0 else out_ap
                    nc.vector.tensor_tensor(dst, zs, ts, op=ALU.mult)

            for rep in range(reps):
              G = [None] * BPC
              for b in range(BPC):
                TR, C = slots[b]
                CL = max(C, TR * 128)
                CHL = _chunks(CL)
                # ---- per-graph loads ----
                raw = p_raw.tile([F0, N], FP, tag='raw')
                nc.sync.dma_start(out=raw, in_=emb_d[b, :, :])
                raw_r = p_raw.tile([F0, N], FP, tag='rawr')
                nc.vector.tensor_copy(raw_r[:, :].bitcast(FR), raw[:, :])
                nrow = p_row.tile([1, N], FP, tag='nrow')
                nc.sync.dma_start(out=nrow, in_=node_d[b, 0:1, :])
                nrow_r = p_row.tile([1, N], FP, tag='nrowr')
                nc.vector.tensor_copy(nrow_r[:, :].bitcast(FR), nrow[:, :])
                nm1 = p_row.tile([1, N], FP, tag='nm1')
                nc.sync.dma_start(out=nm1, in_=node_d[b, 1:2, :])
                ncol = p_sm.tile([128, NT], FP, tag='ncol')
                nc.sync.dma_start(out=ncol, in_=ncol_d[b, :, :])
                invnb = p_sm.tile([128, 1], FP, tag='invnb')
                nc.sync.dma_start(out=invnb, in_=invnb_d[b, :, :])

                # node broadcast (128, CL) via K=1 matmuls
                nbc = p_nbc.tile([128, CMAX], FP, tag='nbc')
                for o, w in CHL:
                    pb = pmi.tile([128, 512], FP, tag='pmi')
                    nc.tensor.matmul(pb[:, 0:w],
                                     lhsT=ones_row_r[0:1, 0:128].bitcast(FR),
                                     rhs=nrow_r[:, o:o + w].bitcast(FR),
                                     start=True, stop=True)
                    nc.vector.tensor_copy(nbc[:, o:o + w], pb[:, 0:w])

                G[b] = dict(raw=raw, raw_r=raw_r, nrow=nrow,
                            nm1=nm1, ncol=ncol, invnb=invnb,
                            nbc=nbc, emb=raw, prev_c=None)
              for ll in range(L):
                for b in range(BPC):
                  TR, C = slots[b]
                  CL = max(C, TR * 128)
                  CH = _chunks(C)
                  CHL = _chunks(CL)
                  st = G[b]
                  raw = st['raw']; raw_r = st['raw_r']
                  nrow = st['nrow']; nm1 = st['nm1']
                  ncol = st['ncol']; invnb = st['invnb']
                  nbc = st['nbc']; emb = st['emb']
                  prev_c = st['prev_c']
                  if 1:
                    first = ll == 0
                    F = F0 if first else 128        # stored feature rows
                    KA = (F0 + 2) if first else 128  # gram contraction depth
                    a1 = F0 if first else 48         # aug row: ones/rB
                    a2 = F0 + 1 if first else 112    # aug row: rA/ones
                    alpha = al[ll]
                    c_l = 45.0 / alpha
                    do_norm = not first

                    T1 = p_T1.tile([KA, CMAX], FP, tag='T1')
                    s_col = p_sm.tile([128, 1], FP, tag='scol')
                    q_col = p_sm.tile([128, 1], FP, tag='qcol')
                    nc.vector.scalar_tensor_tensor(
                        out=T1[0:F, 0:CL].bitcast(FR),
                        in0=emb[:, 0:CL],
                        scalar=(prev_c[0:F, :] if do_norm else 1.0),
                        in1=nbc[0:F, 0:CL],
                        op0=(ALU.subtract if do_norm else ALU.mult),
                        op1=ALU.mult,
                        accum_out=s_col[0:F, :] if do_norm else None)
                    s2 = p_s2.tile([F, CMAX], FP, tag='s2')
                    nc.vector.scalar_tensor_tensor(
                        out=s2[:, 0:CL].bitcast(FR),
                        in0=T1[0:F, 0:CL], scalar=1.0, in1=T1[0:F, 0:CL],
                        op0=ALU.mult, op1=ALU.mult,
                        accum_out=q_col[0:F, :] if do_norm else None)
                    T2 = p_T2.tile([KA, CMAX], FP, tag='T2')
                    nc.vector.tensor_copy(T2[0:F, 0:CL].bitcast(FR),
                                          T1[0:F, 0:CL])

                    # aug rows: rA = -sq/2 ; rB = rA + c_l*(node-1)
                    rA = p_row.tile([1, N], FP, tag='rA')
                    rB = p_row.tile([1, N], FP, tag='rB')
                    for o, w in CHL:
                        sl = slice(o, o + w)
                        pr = pmi.tile([1, 512], FP, tag='pmi')
                        nc.tensor.matmul(
                            pr[:, 0:w],
                            lhsT=ones_col_r[0:F, 0:1].bitcast(FR),
                            rhs=s2[:, sl].bitcast(FR),
                            start=True, stop=True)
                        nc.vector.tensor_scalar(
                            rA[:, sl].bitcast(FR), pr[:, 0:w], -0.5,
                            None, op0=ALU.mult)
                        nc.vector.scalar_tensor_tensor(
                            out=rB[:, sl].bitcast(FR),
                            in0=nm1[:, sl], scalar=c_l,
                            in1=rA[:, sl], op0=ALU.mult, op1=ALU.add)
                    # DMA aug rows (DMA is partition-alignment-free);
                    # chunked so gram c=0 starts before c=1 rows land
                    nc.gpsimd.dma_start(out=T1[a1:a1 + 1, 0:CL].bitcast(FR),
                                        in_=ones_rowN_r[:, 0:CL])
                    nc.sync.dma_start(out=T2[a2:a2 + 1, 0:CL].bitcast(FR),
                                      in_=ones_rowN_r[:, 0:CL].bitcast(FR))
                    for o, w in CHL:
                        sl = slice(o, o + w)
                        nc.gpsimd.dma_start(
                            out=T1[a2:a2 + 1, sl].bitcast(FR), in_=rA[:, sl])
                        nc.sync.dma_start(
                            out=T2[a1:a1 + 1, sl].bitcast(FR),
                            in_=rB[:, sl].bitcast(FR))

                    if do_norm:
                        m = p_sm.tile([128, 1], FP, tag='m')
                        nc.vector.tensor_scalar_mul(m, s_col, invnb[:, 0:1])
                        qn = p_sm.tile([128, 1], FP, tag='qn')
                        nc.vector.tensor_scalar_mul(qn, q_col, invnb[:, 0:1])
                        mm = p_sm.tile([128, 1], FP, tag='mm')
                        nc.vector.tensor_mul(mm, m, m)
                        v = p_sm.tile([128, 1], FP, tag='v')
                        nc.vector.tensor_sub(v, qn, mm)
                        veps = p_sm.tile([128, 1], FP, tag='veps')
                        nc.vector.tensor_scalar(veps, v, EPS, None, op0=ALU.add)
                        inv = p_sm.tile([128, 1], FP, tag='inv')
                        dve_rsqrt(inv[:, :], veps[:, :], 128)
                        EN = p_EN.tile([128, CMAX], FP, tag='EN')
                        nc.vector.tensor_scalar(
                            EN[:, 0:CL].bitcast(FR),
                            T1[0:128, 0:CL], m, inv,
                            op0=ALU.subtract, op1=ALU.mult)
                        ENb = p_EN.tile([128, CMAX], BF, tag='ENb')
                        nc.vector.tensor_copy(ENb[:, 0:CL], EN[:, 0:CL])
                        cat1 = EN
                        wsrc = ENb
                    else:
                        cat1 = raw_r
                        T1b = p_EN.tile([F0, CMAX], BF, tag='ENb')
                        nc.vector.tensor_copy(T1b[:, 0:CL], T1[0:F0, 0:CL])
                        wsrc = T1b  # rows 0:16 = masked raw

                    # gram + wide exp -> adj tiles (adj[t] holds rows t*128..)
                    adj_t = []
                    for t in range(TR):
                        at = p_adj.tile([128, CMAX], BF, tag='adj')
                        adj_t.append(at)
                        pgt = pg.tile([128, CMAX], FP, tag='pg')
                        for o, w in CH:
                            nc.tensor.matmul(
                                pgt[:, o:o + w],
                                lhsT=T2[:, t * 128:(t + 1) * 128].bitcast(FR),
                                rhs=T1[:, o:o + w].bitcast(FR),
                                start=True, stop=True)
                        nc.scalar.activation(
                            at[:, 0:C], pgt[:, 0:C],
                            AF.Exp, bias=0.0, scale=2.0 * alpha)

                    # w blocks (128, WK): transposed masked features + node col
                    WK = 33 if first else 97
                    w_sb = p_w.tile([128, NT * WK], BF, tag='w')
                    if first:
                        for t in range(TR):
                            nc.vector.memset(
                                w_sb[:, t * 33 + 16:t * 33 + 32], 0.0)
                    for t in range(TR):
                        tsl = slice(t * 128, (t + 1) * 128)
                        if first:
                            pt = pmi.tile([128, 512], BF, tag='pmi')
                            nc.tensor.transpose(pt[:, 0:F0], wsrc[0:F0, tsl],
                                                idb[0:F0, 0:F0])
                            nc.vector.tensor_copy(
                                w_sb[:, t * WK:t * WK + F0], pt[:, 0:F0])
                        else:
                            pt = pmi.tile([128, 512], BF, tag='pmi')
                            nc.tensor.transpose(pt[:, 0:H], wsrc[0:H, tsl],
                                                idb[0:H, 0:H])
                            nc.vector.tensor_scalar_mul(
                                w_sb[:, t * WK:t * WK + H],
                                pt[:, 0:H], ncol[:, t:t + 1])
                            pt2 = pmi.tile([128, 512], BF, tag='pmi')
                            nc.tensor.transpose(pt2[:, 0:H],
                                                wsrc[64:112, tsl],
                                                idb[64:112, 0:H])
                            nc.vector.tensor_scalar_mul(
                                w_sb[:, t * WK + H:t * WK + FM],
                                pt2[:, 0:H], ncol[:, t:t + 1])
                        nc.vector.tensor_copy(
                            w_sb[:, t * WK + WK - 1:t * WK + WK],
                            ncol[:, t:t + 1])

                    # part3 (w^T @ adj) -> cat3 + deg row; t-outer so each
                    # stationary w block feeds both column chunks
                    NF3 = F0 if first else FM
                    cat3 = p_c3.tile([NF3, CMAX], FP, tag='c3')
                    degrow = p_row.tile([1, N], FP, tag='degrow')
                    for ci, (o, w) in enumerate(CH):
                        pp_ = pacc.tile([128, 512], FP, tag='acc')
                        for t in range(TR):
                            nc.tensor.matmul(
                                pp_[0:WK, 0:w],
                                lhsT=w_sb[:, t * WK:(t + 1) * WK],
                                rhs=adj_t[t][:, o:o + w],
                                start=(t == 0), stop=(t == TR - 1))
                        nc.vector.tensor_copy(
                            cat3[:, o:o + w].bitcast(FR),
                            pp_[0:NF3, 0:w])
                        nc.vector.tensor_copy(
                            degrow[:, o:o + w].bitcast(FR),
                            pp_[WK - 1:WK, 0:w])

                    # cat2 = cat1 * deg (broadcast deg via K=1 matmul)
                    FC = F0 if first else 128
                    cat2 = p_c2.tile([FC, CMAX], FP, tag='c2')
                    for o, w in CH:
                        sl = slice(o, o + w)
                        pd = pmi.tile([128, 512], FP, tag='pmi')
                        nc.tensor.matmul(
                            pd[0:FC, 0:w],
                            lhsT=ones_row_r[0:1, 0:FC].bitcast(FR),
                            rhs=degrow[:, sl].bitcast(FR),
                            start=True, stop=True)
                        nc.vector.tensor_tensor(
                            cat2[:, sl].bitcast(FR),
                            cat1[:, sl], pd[0:FC, 0:w], op=ALU.mult)

                    # conv+res merged -> emb_out (gapped layout); weights are
                    # zero-padded so gap rows come out exactly 0.
                    # k=1 stationary [Wc1|Wr] (K,128) writes all 128 rows;
                    # k=2 (cat2) and k=3 (cat3) accumulate rows 0:64.
                    emb_out = p_eo.tile([128, CMAX], FP, tag='eo')
                    cacc = p_sm.tile([128, 2], FP, tag='cacc')
                    w1 = wcr0_r if first else wcr_r[:, ll - 1, :]
                    w2 = wc02_r if first else wc2_r[:, ll - 1, :]
                    w3 = wc03_r if first else wc3_r[:, ll - 1, :]
                    # per chunk: conv group (k1 [Wc1|Wr] rows 0:64 only via
                    # its 0:64 columns... keep conv and res in SEPARATE psum
                    # groups (merged-group start/stop over partial partition
                    # ranges is fatal on HW).
                    for ci, (o, w) in enumerate(CH):
                        sl = slice(o, o + w)
                        pc_ = pacc.tile([128, 512], FP, tag='acc')
                        nc.tensor.matmul(
                            pc_[0:64, 0:w], lhsT=w1[:, 0:64].bitcast(FR),
                            rhs=cat1[:, sl].bitcast(FR),
                            start=True, stop=False)
                        nc.tensor.matmul(
                            pc_[0:64, 0:w], lhsT=w2.bitcast(FR),
                            rhs=cat2[:, sl].bitcast(FR),
                            start=False, stop=False)
                        nc.tensor.matmul(
                            pc_[0:64, 0:w], lhsT=w3[:, 0:64].bitcast(FR),
                            rhs=cat3[:, sl].bitcast(FR),
                            start=False, stop=True)
                        pr_ = pacc.tile([128, 512], FP, tag='acc')
                        nc.tensor.matmul(
                            pr_[0:64, 0:w], lhsT=w1[:, 64:128].bitcast(FR),
                            rhs=cat1[:, sl].bitcast(FR),
                            start=True, stop=True)
                        nc.scalar.activation(
                            emb_out[0:64, sl], pc_[0:64, 0:w], AF.Relu,
                            bias=(bc0_sb[:, 0:1] if first
                                  else bc_sb[:, ll - 1:ll]), scale=1.0,
                            accum_out=cacc[0:64, ci:ci + 1])
                        nc.scalar.activation(
                            emb_out[64:128, sl], pr_[0:64, 0:w],
                            AF.Identity,
                            bias=(br0_sb[:, 0:1] if first
                                  else br_sb[:, ll - 1:ll]), scale=1.0,
                            accum_out=cacc[64:128, ci:ci + 1])
                    if CL > C:
                        # next layer's T1 reads cols 0:CL; zero the pad
                        nc.vector.memset(emb_out[:, C:CL], 0.0)
                    emb = emb_out
                    ccol = p_sm.tile([128, 1], FP, tag='ccol')
                    if len(CH) > 1:
                        nc.vector.tensor_tensor(ccol, cacc[:, 0:1],
                                                cacc[:, 1:2], op=ALU.add)
                        nc.vector.tensor_scalar(ccol, ccol, 1.0 / N, None,
                                                op0=ALU.mult)
                    else:
                        nc.vector.tensor_scalar(ccol, cacc[:, 0:1], 1.0 / N,
                                                None, op0=ALU.mult)
                    prev_c = ccol
                  st['emb'] = emb
                  st['prev_c'] = prev_c
              for b in range(BPC):
                TR, C = slots[b]
                st = G[b]
                nbc = st['nbc']; emb = st['emb']
                invnb = st['invnb']
                # ---- epilogue ----
                fm = p_s2.tile([128, CMAX], FP, tag='s2')
                pooled = p_sm.tile([128, 1], FP, tag='pooled')
                nc.vector.scalar_tensor_tensor(
                    out=fm[:, 0:C], in0=emb[:, 0:C], scalar=1.0,
                    in1=nbc[:, 0:C],
                    op0=ALU.mult, op1=ALU.mult, accum_out=pooled)
                p2 = p_sm.tile([128, 1], FP, tag='p2')
                nc.vector.tensor_mul(p2, pooled, pooled)
                pair = p_sm.tile([128, 2], FP, tag='pair')
                nc.vector.tensor_copy(pair[:, 0:1], pooled)
                nc.vector.tensor_copy(pair[:, 1:2], p2)
                psA = pmi.tile([1, 512], FP, tag='pmi')
                nc.tensor.matmul(psA[:, 0:2], lhsT=ones_col[:, 0:1],
                                 rhs=pair, start=True, stop=True)
                psB = pmi.tile([1, 512], FP, tag='pmi')
                nc.tensor.matmul(psB[:, 0:1], lhsT=fclw_sb,
                                 rhs=pooled, start=True, stop=True)
                sc = p_sm.tile([1, 8], FP, tag='sc')
                nc.vector.tensor_scalar(sc[:, 0:1], psA[:, 0:1], 1.0 / FM,
                                        None, op0=ALU.mult)   # mbar
                nc.vector.tensor_scalar(sc[:, 1:2], psA[:, 1:2], 1.0 / FM,
                                        None, op0=ALU.mult)   # qbar
                nc.vector.tensor_mul(sc[:, 2:3], sc[:, 0:1], sc[:, 0:1])
                nc.vector.tensor_sub(sc[:, 3:4], sc[:, 1:2], sc[:, 2:3])  # v
                nc.vector.tensor_scalar(sc[:, 4:5], sc[:, 3:4], EPS, None,
                                        op0=ALU.add)
                dve_rsqrt(sc[:, 5:6], sc[:, 4:5], 1)  # rv
                nc.vector.tensor_scalar_mul(sc[:, 6:7], sc[:, 0:1],
                                            scal_sb[:, 0:1])
                nc.vector.tensor_sub(sc[:, 7:8], psB[:, 0:1], sc[:, 6:7])
                sc2 = p_sm.tile([1, 2], FP, tag='sc2')
                nc.vector.tensor_mul(sc2[:, 0:1], sc[:, 7:8], sc[:, 5:6])
                # sigmoid(z+fb) = 1/(1+exp(-z-fb)); scal[0,2] = -fb
                nc.scalar.activation(sc2[:, 1:2], sc2[:, 0:1], AF.Exp,
                                     bias=scal_sb[:, 2:3], scale=-1.0)
                outsb = p_sm.tile([1, 1], FP, tag='outsb')
                nc.vector.tensor_scalar(outsb, sc2[:, 1:2], 1.0, None,
                                        op0=ALU.add)
                nc.vector.reciprocal(outsb, outsb)
                nc.sync.dma_start(out=out_d[b:b + 1, :], in_=outsb)

    if split_waits:
        _split_multi_waits(nc)
    return nc


_CACHE = {}
_RUN_CACHE = {}
_LAST_INMAPS = None
_LAST_SLOTS = None


def _get_program(alphas, slots):
    key = (tuple(float(a) for a in alphas), slots)
    if key not in _CACHE:
        _CACHE[key] = build_program(alphas, slots)
    return _CACHE[key]


def _gap(a96, axis=0):
    """Gapped-128 feature layout: [0:48]=f[0:48], [64:112]=f[48:96]."""
    shp = list(a96.shape)
    shp[axis] = 128
    out = np.zeros(shp, a96.dtype)
    idx0 = [slice(None)] * a96.ndim
    idx1 = [slice(None)] * a96.ndim
    src0 = [slice(None)] * a96.ndim
    src1 = [slice(None)] * a96.ndim
    idx0[axis] = slice(0, 48); src0[axis] = slice(0, 48)
    idx1[axis] = slice(64, 112); src1[axis] = slice(48, 96)
    out[tuple(idx0)] = a96[tuple(src0)]
    out[tuple(idx1)] = a96[tuple(src1)]
    return out


def kernel(**inputs):
    ins = {k: np.asarray(v) for k, v in inputs.items()}
    emb_in = ins['emb_in'].astype(np.float32)
    adj_mask = ins['adj_mask']
    nb = ins['batch_nb_nodes'].astype(np.float64)
    alphas = ins['alphas'].astype(np.float32)

    node = np.ascontiguousarray(
        np.einsum('bii->bi', adj_mask)).astype(np.float32)       # (B,N)
    noderows = np.ascontiguousarray(
        np.stack([node, node - 1.0], axis=1))                    # (B,2,N)
    node_colm = np.ascontiguousarray(
        node.reshape(B, NT, 128).transpose(0, 2, 1))             # (B,128,NT)
    invnb = np.ascontiguousarray(np.repeat(
        (1.0 / nb).astype(np.float32)[:, None], 128, axis=1)[:, :, None])

    def pad64(a):  # pad last axis H->64 with zeros
        shp = list(a.shape); shp[-1] = 64 - a.shape[-1]
        return np.ascontiguousarray(
            np.concatenate([a, np.zeros(shp, a.dtype)], axis=-1))

    # first layer: Wc0 (48,48) -> T (48,48) = 3 chunks (16,48)
    Wc0T = np.ascontiguousarray(ins['Wc0'].astype(np.float32).T)  # (48,48)
    wc0c = Wc0T.reshape(3, F0, H)
    Wr0T = np.ascontiguousarray(ins['Wr0'].astype(np.float32).T)  # (16,48)
    wcr0 = np.zeros((F0, 128), np.float32)
    wcr0[:, 0:H] = wc0c[0]
    wcr0[:, 64:112] = Wr0T
    wc02 = pad64(np.ascontiguousarray(wc0c[1]))
    wc03 = np.zeros((F0, 128), np.float32)
    wc03[:, 0:H] = wc0c[2]
    bc0 = pad64(ins['bc0'].astype(np.float32).reshape(1, H)).reshape(64, 1)
    br0 = pad64(ins['br0'].astype(np.float32).reshape(1, H)).reshape(64, 1)
    # layer >=1: Wc[l] (48,288) -> T (288,48) -> 3 chunks (96,48);
    # chunk1 merged with Wr into (128,128); chunk2 gapped; chunk3 compact.
    wcr = np.zeros((L - 1, 128, 128), np.float32)
    wc2 = np.zeros((L - 1, 128, 64), np.float32)
    wc3 = np.zeros((L - 1, FM, 128), np.float32)
    for i in range(L - 1):
        WcT = ins['Wc'][i].astype(np.float32).T        # (288,48)
        wcr[i, :, 0:H] = _gap(WcT[0:96])
        wcr[i, :, 64:112] = _gap(ins['Wr'][i].astype(np.float32).T)
        wc2[i, :, 0:H] = _gap(WcT[96:192])
        wc3[i, :, 0:H] = WcT[192:288]
    bc = pad64(ins['bc'].astype(np.float32)).reshape(L - 1, 64, 1)
    br = pad64(ins['br'].astype(np.float32)).reshape(L - 1, 64, 1)
    fclw = _gap(ins['fcl_w'].astype(np.float32).reshape(FM, 1))
    ident = np.zeros((128, 128), np.float32)
    ident[:64, :64] = np.eye(64)
    ident[64:112, 0:48] = np.eye(48)
    fb = float(ins['fcl_b'].reshape(-1)[0])
    scal = np.array([[float(ins['fcl_w'].sum()), fb, -fb, 0.0]], np.float32)

    # sort graphs by nb desc; slot0 = 8 largest, slot1 = 8 smallest
    nbi = ins['batch_nb_nodes'].astype(np.int64)
    order = np.argsort(-nbi, kind='stable')
    slots = tuple(
        (int(-(-int(nbi[order[s * NC]]) // 128)), _width(nbi[order[s * NC]]))
        for s in range(BPC))

    in_maps = []
    for k in range(NC):
        g = [int(order[k]), int(order[NC + k])]
        in_maps.append({
            'emb_in': np.ascontiguousarray(emb_in[g]),
            'noderows': np.ascontiguousarray(noderows[g]),
            'node_colm': np.ascontiguousarray(node_colm[g]),
            'invnb': np.ascontiguousarray(invnb[g]),
            'Wcr0': wcr0, 'Wc02': wc02, 'Wc03': wc03,
            'bc0': bc0, 'br0': br0,
            'Wcr': wcr, 'Wc2': wc2, 'Wc3': wc3, 'bc': bc, 'br': br,
            'fclw': fclw, 'ident': ident, 'scal': scal,
        })

    global _LAST_INMAPS, _LAST_SLOTS
    _LAST_INMAPS = in_maps
    _LAST_SLOTS = slots
    runner = _get_runner(tuple(float(a) for a in alphas), slots)
    outs = runner(in_maps)
    out = np.zeros((B,), np.float32)
    for k in range(NC):
        out[order[k]] = outs[k][0, 0]
        out[order[NC + k]] = outs[k][1, 0]
    return out.astype(np.float32)


def _get_runner(key, slots, reps=1):
    """Persistent jitted SPMD executor (avoids per-call jax retracing)."""
    ck = (key, slots, reps)
    if ck in _RUN_CACHE:
        return _RUN_CACHE[ck]
    import jax
    from jax.experimental.shard_map import shard_map
    from jax.sharding import Mesh, PartitionSpec
    from concourse import bass2jax, mybir as _mb

    pk = (key, slots) if reps == 1 else ck
    if pk not in _CACHE:
        _CACHE[pk] = build_program(list(key), slots, reps)
    nc_prog = _CACHE[pk]
    bass2jax.install_neuronx_cc_hook()

    pname = (nc_prog.partition_id_tensor.name
             if nc_prog.partition_id_tensor else None)
    in_names, out_names, out_avals, zero_outs = [], [], [], []
    for alloc in nc_prog.m.functions[0].allocations:
        if not isinstance(alloc, _mb.MemoryLocationSet):
            continue
        name = alloc.memorylocations[0].name
        if alloc.kind == 'ExternalInput':
            if name != pname:
                in_names.append(name)
        elif alloc.kind == 'ExternalOutput':
            out_names.append(name)
            shape = tuple(alloc.tensor_shape)
            dtype = _mb.dt.np(alloc.dtype)
            out_avals.append(jax.core.ShapedArray(shape, dtype))
            zero_outs.append(np.zeros(shape, dtype))
    n_params = len(in_names)
    all_names = in_names + out_names + ([pname] if pname else [])

    def _body(*args):
        operands = list(args)
        if pname:
            operands.append(bass2jax.partition_id_tensor())
        outs = bass2jax._bass_exec_p.bind(
            *operands, out_avals=tuple(out_avals), in_names=tuple(all_names),
            out_names=tuple(out_names), lowering_input_output_aliases=(),
            sim_require_finite=True, sim_require_nnan=True, nc=nc_prog)
        return tuple(outs)

    devices = jax.devices()[:NC]
    mesh = Mesh(np.asarray(devices), ('core',))
    n_outs = len(out_names)
    sharded = jax.jit(
        shard_map(_body, mesh=mesh,
                  in_specs=(PartitionSpec('core'),) * (n_params + n_outs),
                  out_specs=(PartitionSpec('core'),) * n_outs,
                  check_rep=False),
        keep_unused=True)

    def run(in_maps):
        concat_in = [np.concatenate([np.asarray(m[nm]) for m in in_maps],
                                    axis=0) for nm in in_names]
        concat_zero = [np.zeros((NC * z.shape[0], *z.shape[1:]), z.dtype)
                       for z in zero_outs]
        out_arrs = sharded(*concat_in, *concat_zero)
        o = np.asarray(out_arrs[0]).reshape(NC, *out_avals[0].shape)
        return [o[c] for c in range(NC)]

    _RUN_CACHE[ck] = run
    return run


if __name__ == '__main__':
    sys.path.insert(0, '/root/problem')
    import jax
    import reference as R
    cpu = jax.devices('cpu')[0]
    with jax.default_device(cpu):
        inp = {k: np.asarray(v) for k, v in R.setup_inputs().items()}
        exp = np.asarray(R.reference(**R.setup_inputs()))
    got = kernel(**inp)
    rel = np.abs(got - exp) / (np.abs(exp) + 1e-9)
    print('expected:', exp[:8])
    print('got     :', got[:8])
    print('max rel err:', rel.max())


# revision 29
# speedup vs baseline: 1.0249x; 1.0047x over previous
"""Trainium2 Bass kernel for nn_GCNNSingleKernel (gnn_message_passing).

Strategy:
- Data-parallel over batch B=16 across 8 NeuronCores (2 graphs per core).
- Graphs are SORTED by valid-node count nb and assigned to two per-core
  slots: slot0 gets the 8 largest graphs, slot1 the 8 smallest. Each slot
  is compiled with its own row-tile count T_r = ceil(max_nb/128) and
  column width C (nb rounded up), so all N^2 work (gram matmuls, exp,
  part3) and all N-wide streaming (DVE, conv) shrink to the valid region.
- adj_mask (B,N,N) is outer(node,node); the host extracts the diagonal so
  the 64MB mask never moves to the device.
- Per graph everything stays on-chip. The (N,N) Gaussian kernel matrix is
  produced by one matmul pass with two augmentation rows
  (T1=[emb;1;-sq/2], T2=[emb;-sq/2+BIGNEG*(node-1);1] so T1^T@T2 directly
  yields -d/2 plus a large negative offset on invalid-node columns) and
  one wide ACT Exp pass per 128-row tile. Invalid-node columns die inside
  the exp, so no (N,N) mask multiply is ever needed.
- Features live in a gapped 128-partition layout [conv 0:48 | zeros | res
  64:112 | zeros]; weights are host-padded to match. The residual matmul
  is merged into the first conv matmul (combined (128,128) stationary).
  deg rides as an extra column of the adj@emb^T matmul. Final
  InstanceNorm + fcl + sigmoid run on-device.
"""
import sys
import numpy as np

sys.path.insert(0, '/opt/trn_rl_repo')

import concourse.bass as bass  # noqa: E402
import concourse.tile as tile  # noqa: E402
from concourse import mybir  # noqa: E402

AF = mybir.ActivationFunctionType
ALU = mybir.AluOpType
FP = mybir.dt.float32
FR = mybir.dt.float32r
BF = mybir.dt.bfloat16

B, F0, FM, N, L, H = 16, 16, 96, 1024, 4, 48
NC = 8          # cores
BPC = B // NC   # graphs per core
NT = N // 128   # 8 n-tiles
EPS = 1e-5


def _width(nb):
    """Column width: nb rounded to 64; second 512-chunk kept >=256 wide
    (f32r matmuls below 256 free run at 1/4 rate)."""
    w = min(-(-int(nb) // 64) * 64, N)
    if 512 < w < 768:
        w = 768
    return max(w, 512)


def _chunks(C):
    return [(0, 512), (512, C - 512)] if C > 512 else [(0, C)]


def _split_multi_waits(nc, maxw=1):
    """Walrus (CoreV3) rejects >1 sync-wait on one instruction; spread extras
    onto same-engine NoOps inserted just before."""
    for f in nc.m.functions:
        for bb in f.blocks:
            newlist, changed = [], False
            for inst in bb.instructions:
                si = getattr(inst, 'sync_info', None)
                if si is not None and si.on_wait and len(si.on_wait) > maxw:
                    waits = list(si.on_wait)
                    head, tail = waits[:-maxw], waits[-maxw:]
                    for k in range(0, len(head), maxw):
                        nop = mybir.InstNoOp(
                            name=f'{inst.name}-w{k}', ins=[], outs=[])
                        nop.engine = inst.engine
                        nop.sync_info = mybir.SyncInfo(
                            on_wait=head[k:k + maxw], on_update=[])
                        newlist.append(nop)
                    si.on_wait = tail
                    inst.sync_info = si
                    changed = True
                newlist.append(inst)
            if changed:
                bb.instructions = newlist


def build_program(alphas, slots, reps=1, split_waits=True):
    """Per-core SPMD bass program. alphas: 4 floats baked as immediates.
    slots: ((T_r0, C0), (T_r1, C1)) row-tile count + column width per graph
    slot. reps>1 repeats the whole computation (timing variant).
    split_waits=False keeps multi-waits (CoreSim-friendly; walrus needs True).
    """
    nc = bass.Bass()

    emb_d = nc.dram_tensor('emb_in', [BPC, F0, N], FP, kind='ExternalInput')
    node_d = nc.dram_tensor('noderows', [BPC, 2, N], FP, kind='ExternalInput')
    ncol_d = nc.dram_tensor('node_colm', [BPC, 128, NT], FP, kind='ExternalInput')
    invnb_d = nc.dram_tensor('invnb', [BPC, 128, 1], FP, kind='ExternalInput')
    # first layer: k1+res merged stationary (16,128); k2 (16,64); k3 (16,64)
    wcr0_d = nc.dram_tensor('Wcr0', [F0, 128], FP, kind='ExternalInput')
    wc02_d = nc.dram_tensor('Wc02', [F0, 64], FP, kind='ExternalInput')
    wc03_d = nc.dram_tensor('Wc03', [F0, 128], FP, kind='ExternalInput')
    bc0_d = nc.dram_tensor('bc0', [64, 1], FP, kind='ExternalInput')
    br0_d = nc.dram_tensor('br0', [64, 1], FP, kind='ExternalInput')
    # layers 1..3: merged k1+res (128,128); k2 gapped (128,64); k3 (96,64)
    wcr_d = nc.dram_tensor('Wcr', [L - 1, 128, 128], FP, kind='ExternalInput')
    wc2_d = nc.dram_tensor('Wc2', [L - 1, 128, 64], FP, kind='ExternalInput')
    wc3_d = nc.dram_tensor('Wc3', [L - 1, FM, 128], FP, kind='ExternalInput')
    bc_d = nc.dram_tensor('bc', [L - 1, 64, 1], FP, kind='ExternalInput')
    br_d = nc.dram_tensor('br', [L - 1, 64, 1], FP, kind='ExternalInput')
    fclw_d = nc.dram_tensor('fclw', [128, 1], FP, kind='ExternalInput')
    id_d = nc.dram_tensor('ident', [128, 128], FP, kind='ExternalInput')
    scal_d = nc.dram_tensor('scal', [1, 4], FP, kind='ExternalInput')
    out_d = nc.dram_tensor('out', [BPC, 1], FP, kind='ExternalOutput')

    al = [float(a) for a in alphas]
    # n-side tensors (T1/T2/s2/EN/nbc/rA/rB) must span the full row-tile
    # range CL = max(C, TR*128); m-side streaming stays at C.
    CMAX = max(max(s[1], s[0] * 128) for s in slots)

    with tile.TileContext(nc) as tc:
        from contextlib import ExitStack
        with ExitStack() as ctx:
            const = ctx.enter_context(tc.tile_pool(name='const', bufs=1))
            p_raw = ctx.enter_context(tc.tile_pool(name='raw', bufs=2))
            p_nbc = ctx.enter_context(tc.tile_pool(name='nbc', bufs=2))
            p_T1 = ctx.enter_context(tc.tile_pool(name='T1', bufs=2))
            p_T2 = ctx.enter_context(tc.tile_pool(name='T2', bufs=2))
            p_s2 = ctx.enter_context(tc.tile_pool(name='s2', bufs=2))
            p_EN = ctx.enter_context(tc.tile_pool(name='EN', bufs=2))
            p_adj = ctx.enter_context(tc.tile_pool(name='adj', bufs=12))
            p_w = ctx.enter_context(tc.tile_pool(name='w', bufs=2))
            p_c2 = ctx.enter_context(tc.tile_pool(name='c2', bufs=2))
            p_c3 = ctx.enter_context(tc.tile_pool(name='c3', bufs=2))
            p_eo = ctx.enter_context(tc.tile_pool(name='eo', bufs=2))
            p_sm = ctx.enter_context(tc.tile_pool(name='sm', bufs=2))
            p_row = ctx.enter_context(tc.tile_pool(name='row', bufs=2))
            # PSUM: pg 2x(128,CMAX)=4 banks; acc 2x(128,512)=2; pmi 2x=2
            pg = ctx.enter_context(tc.tile_pool(name='ps_g', bufs=2, space='PSUM'))
            pacc = ctx.enter_context(tc.tile_pool(name='ps_a', bufs=2, space='PSUM'))
            pmi = ctx.enter_context(tc.tile_pool(name='ps_mi', bufs=2, space='PSUM'))

            # ---- constants ----
            id_sb = const.tile([128, 128], FP)
            nc.sync.dma_start(out=id_sb, in_=id_d[:, :])
            idb = const.tile([128, 128], BF)
            nc.vector.tensor_copy(idb, id_sb)
            ones_col = const.tile([128, 1], FP)
            nc.vector.memset(ones_col, 1.0)
            ones_row = const.tile([1, 128], FP)
            nc.vector.memset(ones_row, 1.0)
            ones_rowN = const.tile([1, N], FP)
            nc.vector.memset(ones_rowN, 1.0)
            wcr0_sb = const.tile([F0, 128], FP)
            nc.sync.dma_start(out=wcr0_sb, in_=wcr0_d[:, :])
            wc02_sb = const.tile([F0, 64], FP)
            nc.sync.dma_start(out=wc02_sb, in_=wc02_d[:, :])
            wc03_sb = const.tile([F0, 128], FP)
            nc.sync.dma_start(out=wc03_sb, in_=wc03_d[:, :])
            bc0_sb = const.tile([64, 1], FP)
            nc.sync.dma_start(out=bc0_sb, in_=bc0_d[:, :])
            br0_sb = const.tile([64, 1], FP)
            nc.sync.dma_start(out=br0_sb, in_=br0_d[:, :])
            wcr_sb = const.tile([128, L - 1, 128], FP)
            for ll in range(L - 1):
                nc.sync.dma_start(out=wcr_sb[:, ll, :], in_=wcr_d[ll, :, :])
            wc2_sb = const.tile([128, L - 1, 64], FP)
            for ll in range(L - 1):
                nc.sync.dma_start(out=wc2_sb[:, ll, :], in_=wc2_d[ll, :, :])
            wc3_sb = const.tile([FM, L - 1, 128], FP)
            for ll in range(L - 1):
                nc.sync.dma_start(out=wc3_sb[:, ll, :], in_=wc3_d[ll, :, :])
            bc_sb = const.tile([64, L - 1], FP)
            for ll in range(L - 1):
                nc.sync.dma_start(out=bc_sb[:, ll:ll + 1], in_=bc_d[ll, :, :])
            br_sb = const.tile([64, L - 1], FP)
            for ll in range(L - 1):
                nc.sync.dma_start(out=br_sb[:, ll:ll + 1], in_=br_d[ll, :, :])
            fclw_sb = const.tile([128, 1], FP)
            nc.sync.dma_start(out=fclw_sb, in_=fclw_d[:, :])
            scal_sb = const.tile([1, 4], FP)
            nc.sync.dma_start(out=scal_sb, in_=scal_d[:, :])
            # f32r-rounded weight copies (PE runs f32r matmuls single-pass)
            wcr0_r = const.tile([F0, 128], FP)
            nc.vector.tensor_copy(wcr0_r[:, :].bitcast(FR), wcr0_sb[:, :])
            wc02_r = const.tile([F0, 64], FP)
            nc.vector.tensor_copy(wc02_r[:, :].bitcast(FR), wc02_sb[:, :])
            wc03_r = const.tile([F0, 128], FP)
            nc.vector.tensor_copy(wc03_r[:, :].bitcast(FR), wc03_sb[:, :])
            wcr_r = const.tile([128, L - 1, 128], FP)
            nc.vector.tensor_copy(wcr_r[:, :, :].bitcast(FR),
                                  wcr_sb[:, :, :])
            wc2_r = const.tile([128, L - 1, 64], FP)
            nc.vector.tensor_copy(wc2_r[:, :, :].bitcast(FR), wc2_sb[:, :, :])
            wc3_r = const.tile([FM, L - 1, 128], FP)
            nc.vector.tensor_copy(wc3_r[:, :, :].bitcast(FR), wc3_sb[:, :, :])
            ones_row_r = const.tile([1, 128], FP)
            nc.vector.tensor_copy(ones_row_r[:, :].bitcast(FR), ones_row[:, :])
            zer16 = const.tile([128, 16], FP)
            nc.vector.memset(zer16, 0.0)
            ones_rowN_r = const.tile([1, N], FP)
            nc.vector.tensor_copy(ones_rowN_r[:, :].bitcast(FR),
                                  ones_rowN[:, :])
            ones_col_r = const.tile([128, 1], FP)
            nc.vector.tensor_copy(ones_col_r[:, :].bitcast(FR), ones_col[:, :])
            magic = const.tile([128, 1], mybir.dt.uint32)
            nc.vector.memset(magic, 0x5f3759df)

            def dve_rsqrt(out_ap, a_ap, np_):
                """out = 1/sqrt(a) via magic seed + 2 Newton iters (DVE only)."""
                U32 = mybir.dt.uint32
                z = p_sm.tile([128, 1], FP, tag='rsq_z')
                t = p_sm.tile([128, 1], FP, tag='rsq_t')
                zs = z[0:np_, :]
                ts = t[0:np_, :]
                nc.vector.tensor_scalar(zs.bitcast(U32), a_ap.bitcast(U32), 1,
                                        None, op0=ALU.logical_shift_right)
                nc.vector.scalar_tensor_tensor(
                    out=zs.bitcast(U32), in0=magic[0:np_, :], scalar=0,
                    in1=zs.bitcast(U32), op0=ALU.bypass, op1=ALU.subtract)
                for it in range(2):
                    nc.vector.tensor_tensor(ts, zs, zs, op=ALU.mult)
                    nc.vector.tensor_tensor(ts, ts, a_ap, op=ALU.mult)
                    nc.vector.tensor_scalar(ts, ts, -0.5, 1.5, op0=ALU.mult,
                                            op1=ALU.add)
                    dst = zs if it == 0 else out_ap
                    nc.vector.tensor_tensor(dst, zs, ts, op=ALU.mult)

            for rep in range(reps):
              G = [None] * BPC
              for b in range(BPC):
                TR, C = slots[b]
                CL = max(C, TR * 128)
                CHL = _chunks(CL)
                # ---- per-graph loads ----
                raw = p_raw.tile([F0, N], FP, tag='raw')
                nc.sync.dma_start(out=raw, in_=emb_d[b, :, :])
                raw_r = p_raw.tile([F0, N], FP, tag='rawr')
                nc.vector.tensor_copy(raw_r[:, :].bitcast(FR), raw[:, :])
                nrow = p_row.tile([1, N], FP, tag='nrow')
                nc.sync.dma_start(out=nrow, in_=node_d[b, 0:1, :])
                nrow_r = p_row.tile([1, N], FP, tag='nrowr')
                nc.vector.tensor_copy(nrow_r[:, :].bitcast(FR), nrow[:, :])
                nm1 = p_row.tile([1, N], FP, tag='nm1')
                nc.sync.dma_start(out=nm1, in_=node_d[b, 1:2, :])
                ncol = p_sm.tile([128, NT], FP, tag='ncol')
                nc.sync.dma_start(out=ncol, in_=ncol_d[b, :, :])
                invnb = p_sm.tile([128, 1], FP, tag='invnb')
                nc.sync.dma_start(out=invnb, in_=invnb_d[b, :, :])

                # node broadcast (128, CL) via K=1 matmuls
                nbc = p_nbc.tile([128, CMAX], FP, tag='nbc')
                for o, w in CHL:
                    pb = pmi.tile([128, 512], FP, tag='pmi')
                    nc.tensor.matmul(pb[:, 0:w],
                                     lhsT=ones_row_r[0:1, 0:128].bitcast(FR),
                                     rhs=nrow_r[:, o:o + w].bitcast(FR),
                                     start=True, stop=True)
                    nc.vector.tensor_copy(nbc[:, o:o + w], pb[:, 0:w])

                G[b] = dict(raw=raw, raw_r=raw_r, nrow=nrow,
                            nm1=nm1, ncol=ncol, invnb=invnb,
                            nbc=nbc, emb=raw, prev_c=None)
              for ll in range(L):
                for b in range(BPC):
                  TR, C = slots[b]
                  CL = max(C, TR * 128)
                  CH = _chunks(C)
                  CHL = _chunks(CL)
                  st = G[b]
                  raw = st['raw']; raw_r = st['raw_r']
                  nrow = st['nrow']; nm1 = st['nm1']
                  ncol = st['ncol']; invnb = st['invnb']
                  nbc = st['nbc']; emb = st['emb']
                  prev_c = st['prev_c']
                  if 1:
                    first = ll == 0
                    F = F0 if first else 128        # stored feature rows
                    KA = (F0 + 2) if first else 128  # gram contraction depth
                    a1 = F0 if first else 48         # aug row: ones/rB
                    a2 = F0 + 1 if first else 112    # aug row: rA/ones
                    alpha = al[ll]
                    c_l = 45.0 / alpha
                    do_norm = not first

                    T1 = p_T1.tile([KA, CMAX], FP, tag='T1')
                    s_col = p_sm.tile([128, 1], FP, tag='scol')
                    q_col = p_sm.tile([128, 1], FP, tag='qcol')
                    nc.vector.scalar_tensor_tensor(
                        out=T1[0:F, 0:CL].bitcast(FR),
                        in0=emb[:, 0:CL],
                        scalar=(prev_c[0:F, :] if do_norm else 1.0),
                        in1=nbc[0:F, 0:CL],
                        op0=(ALU.subtract if do_norm else ALU.mult),
                        op1=ALU.mult,
                        accum_out=s_col[0:F, :] if do_norm else None)
                    s2 = p_s2.tile([F, CMAX], FP, tag='s2')
                    nc.vector.scalar_tensor_tensor(
                        out=s2[:, 0:CL].bitcast(FR),
                        in0=T1[0:F, 0:CL], scalar=1.0, in1=T1[0:F, 0:CL],
                        op0=ALU.mult, op1=ALU.mult,
                        accum_out=q_col[0:F, :] if do_norm else None)
                    T2 = p_T2.tile([KA, CMAX], FP, tag='T2')
                    nc.vector.tensor_copy(T2[0:F, 0:CL].bitcast(FR),
                                          T1[0:F, 0:CL])

                    # aug rows: rA = -sq/2 ; rB = rA + c_l*(node-1)
                    rA = p_row.tile([1, N], FP, tag='rA')
                    rB = p_row.tile([1, N], FP, tag='rB')
                    for o, w in CHL:
                        sl = slice(o, o + w)
                        pr = pmi.tile([1, 512], FP, tag='pmi')
                        nc.tensor.matmul(
                            pr[:, 0:w],
                            lhsT=ones_col_r[0:F, 0:1].bitcast(FR),
                            rhs=s2[:, sl].bitcast(FR),
                            start=True, stop=True)
                        nc.vector.tensor_scalar(
                            rA[:, sl].bitcast(FR), pr[:, 0:w], -0.5,
                            None, op0=ALU.mult)
                        nc.vector.scalar_tensor_tensor(
                            out=rB[:, sl].bitcast(FR),
                            in0=nm1[:, sl], scalar=c_l,
                            in1=rA[:, sl], op0=ALU.mult, op1=ALU.add)
                    # DMA aug rows (DMA is partition-alignment-free);
                    # chunked so gram c=0 starts before c=1 rows land
                    nc.gpsimd.dma_start(out=T1[a1:a1 + 1, 0:CL].bitcast(FR),
                                        in_=ones_rowN_r[:, 0:CL])
                    nc.sync.dma_start(out=T2[a2:a2 + 1, 0:CL].bitcast(FR),
                                      in_=ones_rowN_r[:, 0:CL].bitcast(FR))
                    for o, w in CHL:
                        sl = slice(o, o + w)
                        nc.gpsimd.dma_start(
                            out=T1[a2:a2 + 1, sl].bitcast(FR), in_=rA[:, sl])
                        nc.sync.dma_start(
                            out=T2[a1:a1 + 1, sl].bitcast(FR),
                            in_=rB[:, sl].bitcast(FR))

                    if do_norm:
                        m = p_sm.tile([128, 1], FP, tag='m')
                        nc.vector.tensor_scalar_mul(m, s_col, invnb[:, 0:1])
                        qn = p_sm.tile([128, 1], FP, tag='qn')
                        nc.vector.tensor_scalar_mul(qn, q_col, invnb[:, 0:1])
                        mm = p_sm.tile([128, 1], FP, tag='mm')
                        nc.vector.tensor_mul(mm, m, m)
                        v = p_sm.tile([128, 1], FP, tag='v')
                        nc.vector.tensor_sub(v, qn, mm)
                        veps = p_sm.tile([128, 1], FP, tag='veps')
                        nc.vector.tensor_scalar(veps, v, EPS, None, op0=ALU.add)
                        inv = p_sm.tile([128, 1], FP, tag='inv')
                        dve_rsqrt(inv[:, :], veps[:, :], 128)
                        EN = p_EN.tile([128, CMAX], FP, tag='EN')
                        nc.vector.tensor_scalar(
                            EN[:, 0:CL].bitcast(FR),
                            T1[0:128, 0:CL], m, inv,
                            op0=ALU.subtract, op1=ALU.mult)
                        ENb = p_EN.tile([128, CMAX], BF, tag='ENb')
                        nc.vector.tensor_copy(ENb[:, 0:CL], EN[:, 0:CL])
                        cat1 = EN
                        wsrc = ENb
                    else:
                        cat1 = raw_r
                        T1b = p_EN.tile([F0, CMAX], BF, tag='ENb')
                        nc.vector.tensor_copy(T1b[:, 0:CL], T1[0:F0, 0:CL])
                        wsrc = T1b  # rows 0:16 = masked raw

                    # gram + wide exp -> adj tiles (adj[t] holds rows t*128..)
                    adj_t = []
                    for t in range(TR):
                        at = p_adj.tile([128, CMAX], BF, tag='adj')
                        adj_t.append(at)
                        pgt = pg.tile([128, CMAX], FP, tag='pg')
                        for o, w in CH:
                            nc.tensor.matmul(
                                pgt[:, o:o + w],
                                lhsT=T2[:, t * 128:(t + 1) * 128].bitcast(FR),
                                rhs=T1[:, o:o + w].bitcast(FR),
                                start=True, stop=True)
                        nc.scalar.activation(
                            at[:, 0:C], pgt[:, 0:C],
                            AF.Exp, bias=0.0, scale=2.0 * alpha)

                    # w blocks (128, WK): transposed masked features + node col
                    WK = 33 if first else 97
                    w_sb = p_w.tile([128, NT * WK], BF, tag='w')
                    if first:
                        for t in range(TR):
                            nc.vector.memset(
                                w_sb[:, t * 33 + 16:t * 33 + 32], 0.0)
                    for t in range(TR):
                        tsl = slice(t * 128, (t + 1) * 128)
                        if first:
                            pt = pmi.tile([128, 512], BF, tag='pmi')
                            nc.tensor.transpose(pt[:, 0:F0], wsrc[0:F0, tsl],
                                                idb[0:F0, 0:F0])
                            nc.vector.tensor_copy(
                                w_sb[:, t * WK:t * WK + F0], pt[:, 0:F0])
                        else:
                            pt = pmi.tile([128, 512], BF, tag='pmi')
                            nc.tensor.transpose(pt[:, 0:H], wsrc[0:H, tsl],
                                                idb[0:H, 0:H])
                            nc.vector.tensor_scalar_mul(
                                w_sb[:, t * WK:t * WK + H],
                                pt[:, 0:H], ncol[:, t:t + 1])
                            pt2 = pmi.tile([128, 512], BF, tag='pmi')
                            nc.tensor.transpose(pt2[:, 0:H],
                                                wsrc[64:112, tsl],
                                                idb[64:112, 0:H])
                            nc.vector.tensor_scalar_mul(
                                w_sb[:, t * WK + H:t * WK + FM],
                                pt2[:, 0:H], ncol[:, t:t + 1])
                        nc.vector.tensor_copy(
                            w_sb[:, t * WK + WK - 1:t * WK + WK],
                            ncol[:, t:t + 1])

                    # part3 (w^T @ adj) -> cat3 + deg row; t-outer so each
                    # stationary w block feeds both column chunks
                    NF3 = F0 if first else FM
                    cat3 = p_c3.tile([NF3, CMAX], FP, tag='c3')
                    degrow = p_row.tile([1, N], FP, tag='degrow')
                    for ci, (o, w) in enumerate(CH):
                        pp_ = pacc.tile([128, 512], FP, tag='acc')
                        for t in range(TR):
                            nc.tensor.matmul(
                                pp_[0:WK, 0:w],
                                lhsT=w_sb[:, t * WK:(t + 1) * WK],
                                rhs=adj_t[t][:, o:o + w],
                                start=(t == 0), stop=(t == TR - 1))
                        nc.vector.tensor_copy(
                            cat3[:, o:o + w].bitcast(FR),
                            pp_[0:NF3, 0:w])
                        nc.vector.tensor_copy(
                            degrow[:, o:o + w].bitcast(FR),
                            pp_[WK - 1:WK, 0:w])

                    # cat2 = cat1 * deg (broadcast deg via K=1 matmul)
                    FC = F0 if first else 128
                    cat2 = p_c2.tile([FC, CMAX], FP, tag='c2')
                    for o, w in CH:
                        sl = slice(o, o + w)
                        pd = pmi.tile([128, 512], FP, tag='pmi')
                        nc.tensor.matmul(
                            pd[0:FC, 0:w],
                            lhsT=ones_row_r[0:1, 0:FC].bitcast(FR),
                            rhs=degrow[:, sl].bitcast(FR),
                            start=True, stop=True)
                        nc.vector.tensor_tensor(
                            cat2[:, sl].bitcast(FR),
                            cat1[:, sl], pd[0:FC, 0:w], op=ALU.mult)

                    # conv+res merged -> emb_out (gapped layout); weights are
                    # zero-padded so gap rows come out exactly 0.
                    # k=1 stationary [Wc1|Wr] (K,128) writes all 128 rows;
                    # k=2 (cat2) and k=3 (cat3) accumulate rows 0:64.
                    emb_out = p_eo.tile([128, CMAX], FP, tag='eo')
                    cacc = p_sm.tile([128, 2], FP, tag='cacc')
                    w1 = wcr0_r if first else wcr_r[:, ll - 1, :]
                    w2 = wc02_r if first else wc2_r[:, ll - 1, :]
                    w3 = wc03_r if first else wc3_r[:, ll - 1, :]
                    # per chunk: conv group (k1 [Wc1|Wr] rows 0:64 only via
                    # its 0:64 columns... keep conv and res in SEPARATE psum
                    # groups (merged-group start/stop over partial partition
                    # ranges is fatal on HW).
                    for ci, (o, w) in enumerate(CH):
                        sl = slice(o, o + w)
                        pc_ = pacc.tile([128, 512], FP, tag='acc')
                        nc.tensor.matmul(
                            pc_[0:64, 0:w], lhsT=w1[:, 0:64].bitcast(FR),
                            rhs=cat1[:, sl].bitcast(FR),
                            start=True, stop=False)
                        nc.tensor.matmul(
                            pc_[0:64, 0:w], lhsT=w2.bitcast(FR),
                            rhs=cat2[:, sl].bitcast(FR),
                            start=False, stop=False)
                        nc.tensor.matmul(
                            pc_[0:64, 0:w], lhsT=w3[:, 0:64].bitcast(FR),
                            rhs=cat3[:, sl].bitcast(FR),
                            start=False, stop=True)
                        pr_ = pacc.tile([128, 512], FP, tag='acc')
                        nc.tensor.matmul(
                            pr_[0:64, 0:w], lhsT=w1[:, 64:128].bitcast(FR),
                            rhs=cat1[:, sl].bitcast(FR),
                            start=True, stop=True)
                        nc.scalar.activation(
                            emb_out[0:64, sl], pc_[0:64, 0:w], AF.Relu,
                            bias=(bc0_sb[:, 0:1] if first
                                  else bc_sb[:, ll - 1:ll]), scale=1.0,
                            accum_out=cacc[0:64, ci:ci + 1])
                        nc.scalar.activation(
                            emb_out[64:128, sl], pr_[0:64, 0:w],
                            AF.Identity,
                            bias=(br0_sb[:, 0:1] if first
                                  else br_sb[:, ll - 1:ll]), scale=1.0,
                            accum_out=cacc[64:128, ci:ci + 1])
                    if CL > C:
                        # next layer's T1 reads cols 0:CL; zero the pad
                        nc.vector.memset(emb_out[:, C:CL], 0.0)
                    emb = emb_out
                    ccol = p_sm.tile([128, 1], FP, tag='ccol')
                    if len(CH) > 1:
                        nc.vector.tensor_tensor(ccol, cacc[:, 0:1],
                                                cacc[:, 1:2], op=ALU.add)
                        nc.vector.tensor_scalar(ccol, ccol, 1.0 / N, None,
                                                op0=ALU.mult)
                    else:
                        nc.vector.tensor_scalar(ccol, cacc[:, 0:1], 1.0 / N,
                                                None, op0=ALU.mult)
                    prev_c = ccol
                  st['emb'] = emb
                  st['prev_c'] = prev_c
              for b in range(BPC):
                TR, C = slots[b]
                st = G[b]
                nbc = st['nbc']; emb = st['emb']
                invnb = st['invnb']
                # ---- epilogue ----
                fm = p_s2.tile([128, CMAX], FP, tag='s2')
                pooled = p_sm.tile([128, 1], FP, tag='pooled')
                nc.vector.scalar_tensor_tensor(
                    out=fm[:, 0:C], in0=emb[:, 0:C], scalar=1.0,
                    in1=nbc[:, 0:C],
                    op0=ALU.mult, op1=ALU.mult, accum_out=pooled)
                p2 = p_sm.tile([128, 1], FP, tag='p2')
                nc.vector.tensor_mul(p2, pooled, pooled)
                pair = p_sm.tile([128, 2], FP, tag='pair')
                nc.vector.tensor_copy(pair[:, 0:1], pooled)
                nc.vector.tensor_copy(pair[:, 1:2], p2)
                psA = pmi.tile([1, 512], FP, tag='pmi')
                nc.tensor.matmul(psA[:, 0:2], lhsT=ones_col[:, 0:1],
                                 rhs=pair, start=True, stop=True)
                psB = pmi.tile([1, 512], FP, tag='pmi')
                nc.tensor.matmul(psB[:, 0:1], lhsT=fclw_sb,
                                 rhs=pooled, start=True, stop=True)
                sc = p_sm.tile([1, 8], FP, tag='sc')
                nc.vector.tensor_scalar(sc[:, 0:1], psA[:, 0:1], 1.0 / FM,
                                        None, op0=ALU.mult)   # mbar
                nc.vector.tensor_scalar(sc[:, 1:2], psA[:, 1:2], 1.0 / FM,
                                        None, op0=ALU.mult)   # qbar
                nc.vector.tensor_mul(sc[:, 2:3], sc[:, 0:1], sc[:, 0:1])
                nc.vector.tensor_sub(sc[:, 3:4], sc[:, 1:2], sc[:, 2:3])  # v
                nc.vector.tensor_scalar(sc[:, 4:5], sc[:, 3:4], EPS, None,
                                        op0=ALU.add)
                dve_rsqrt(sc[:, 5:6], sc[:, 4:5], 1)  # rv
                nc.vector.tensor_scalar_mul(sc[:, 6:7], sc[:, 0:1],
                                            scal_sb[:, 0:1])
                nc.vector.tensor_sub(sc[:, 7:8], psB[:, 0:1], sc[:, 6:7])
                sc2 = p_sm.tile([1, 2], FP, tag='sc2')
                nc.vector.tensor_mul(sc2[:, 0:1], sc[:, 7:8], sc[:, 5:6])
                # sigmoid(z+fb) = 1/(1+exp(-z-fb)); scal[0,2] = -fb
                nc.scalar.activation(sc2[:, 1:2], sc2[:, 0:1], AF.Exp,
                                     bias=scal_sb[:, 2:3], scale=-1.0)
                outsb = p_sm.tile([1, 1], FP, tag='outsb')
                nc.vector.tensor_scalar(outsb, sc2[:, 1:2], 1.0, None,
                                        op0=ALU.add)
                nc.vector.reciprocal(outsb, outsb)
                nc.sync.dma_start(out=out_d[b:b + 1, :], in_=outsb)

    if split_waits:
        _split_multi_waits(nc)
    return nc


_CACHE = {}
_RUN_CACHE = {}
_LAST_INMAPS = None
_LAST_SLOTS = None


def _get_program(alphas, slots):
    key = (tuple(float(a) for a in alphas), slots)
    if key not in _CACHE:
        _CACHE[key] = build_program(alphas, slots)
    return _CACHE[key]


def _gap(a96, axis=0):
    """Gapped-128 feature layout: [0:48]=f[0:48], [64:112]=f[48:96]."""
    shp = list(a96.shape)
    shp[axis] = 128
    out = np.zeros(shp, a96.dtype)
    idx0 = [slice(None)] * a96.ndim
    idx1 = [slice(None)] * a96.ndim
    src0 = [slice(None)] * a96.ndim
    src1 = [slice(None)] * a96.ndim
    idx0[axis] = slice(0, 48); src0[axis] = slice(0, 48)
    idx1[axis] = slice(64, 112); src1[axis] = slice(48, 96)
    out[tuple(idx0)] = a96[tuple(src0)]
    out[tuple(idx1)] = a96[tuple(src1)]
    return out


def kernel(**inputs):
    ins = {k: np.asarray(v) for k, v in inputs.items()}
    emb_in = ins['emb_in'].astype(np.float32)
    adj_mask = ins['adj_mask']
    nb = ins['batch_nb_nodes'].astype(np.float64)
    alphas = ins['alphas'].astype(np.float32)

    node = np.ascontiguousarray(
        np.einsum('bii->bi', adj_mask)).astype(np.float32)       # (B,N)
    noderows = np.ascontiguousarray(
        np.stack([node, node - 1.0], axis=1))                    # (B,2,N)
    node_colm = np.ascontiguousarray(
        node.reshape(B, NT, 128).transpose(0, 2, 1))             # (B,128,NT)
    invnb = np.ascontiguousarray(np.repeat(
        (1.0 / nb).astype(np.float32)[:, None], 128, axis=1)[:, :, None])

    def pad64(a):  # pad last axis H->64 with zeros
        shp = list(a.shape); shp[-1] = 64 - a.shape[-1]
        return np.ascontiguousarray(
            np.concatenate([a, np.zeros(shp, a.dtype)], axis=-1))

    # first layer: Wc0 (48,48) -> T (48,48) = 3 chunks (16,48)
    Wc0T = np.ascontiguousarray(ins['Wc0'].astype(np.float32).T)  # (48,48)
    wc0c = Wc0T.reshape(3, F0, H)
    Wr0T = np.ascontiguousarray(ins['Wr0'].astype(np.float32).T)  # (16,48)
    wcr0 = np.zeros((F0, 128), np.float32)
    wcr0[:, 0:H] = wc0c[0]
    wcr0[:, 64:112] = Wr0T
    wc02 = pad64(np.ascontiguousarray(wc0c[1]))
    wc03 = np.zeros((F0, 128), np.float32)
    wc03[:, 0:H] = wc0c[2]
    bc0 = pad64(ins['bc0'].astype(np.float32).reshape(1, H)).reshape(64, 1)
    br0 = pad64(ins['br0'].astype(np.float32).reshape(1, H)).reshape(64, 1)
    # layer >=1: Wc[l] (48,288) -> T (288,48) -> 3 chunks (96,48);
    # chunk1 merged with Wr into (128,128); chunk2 gapped; chunk3 compact.
    wcr = np.zeros((L - 1, 128, 128), np.float32)
    wc2 = np.zeros((L - 1, 128, 64), np.float32)
    wc3 = np.zeros((L - 1, FM, 128), np.float32)
    for i in range(L - 1):
        WcT = ins['Wc'][i].astype(np.float32).T        # (288,48)
        wcr[i, :, 0:H] = _gap(WcT[0:96])
        wcr[i, :, 64:112] = _gap(ins['Wr'][i].astype(np.float32).T)
        wc2[i, :, 0:H] = _gap(WcT[96:192])
        wc3[i, :, 0:H] = WcT[192:288]
    bc = pad64(ins['bc'].astype(np.float32)).reshape(L - 1, 64, 1)
    br = pad64(ins['br'].astype(np.float32)).reshape(L - 1, 64, 1)
    fclw = _gap(ins['fcl_w'].astype(np.float32).reshape(FM, 1))
    ident = np.zeros((128, 128), np.float32)
    ident[:64, :64] = np.eye(64)
    ident[64:112, 0:48] = np.eye(48)
    fb = float(ins['fcl_b'].reshape(-1)[0])
    scal = np.array([[float(ins['fcl_w'].sum()), fb, -fb, 0.0]], np.float32)

    # sort graphs by nb desc; slot0 = 8 largest, slot1 = 8 smallest
    nbi = ins['batch_nb_nodes'].astype(np.int64)
    order = np.argsort(-nbi, kind='stable')
    slots = tuple(
        (int(-(-int(nbi[order[s * NC]]) // 128)), _width(nbi[order[s * NC]]))
        for s in range(BPC))

    in_maps = []
    for k in range(NC):
        g = [int(order[k]), int(order[NC + k])]
        in_maps.append({
            'emb_in': np.ascontiguousarray(emb_in[g]),
            'noderows': np.ascontiguousarray(noderows[g]),
            'node_colm': np.ascontiguousarray(node_colm[g]),
            'invnb': np.ascontiguousarray(invnb[g]),
            'Wcr0': wcr0, 'Wc02': wc02, 'Wc03': wc03,
            'bc0': bc0, 'br0': br0,
            'Wcr': wcr, 'Wc2': wc2, 'Wc3': wc3, 'bc': bc, 'br': br,
            'fclw': fclw, 'ident': ident, 'scal': scal,
        })

    global _LAST_INMAPS, _LAST_SLOTS
    _LAST_INMAPS = in_maps
    _LAST_SLOTS = slots
    runner = _get_runner(tuple(float(a) for a in alphas), slots)
    outs = runner(in_maps)
    out = np.zeros((B,), np.float32)
    for k in range(NC):
        out[order[k]] = outs[k][0, 0]
        out[order[NC + k]] = outs[k][1, 0]
    return out.astype(np.float32)


def _get_runner(key, slots, reps=1):
    """Persistent jitted SPMD executor (avoids per-call jax retracing)."""
    ck = (key, slots, reps)
    if ck in _RUN_CACHE:
        return _RUN_CACHE[ck]
    import jax
    from jax.experimental.shard_map import shard_map
    from jax.sharding import Mesh, PartitionSpec
    from concourse import bass2jax, mybir as _mb

    pk = (key, slots) if reps == 1 else ck
    if pk not in _CACHE:
        _CACHE[pk] = build_program(list(key), slots, reps)
    nc_prog = _CACHE[pk]
    bass2jax.install_neuronx_cc_hook()

    pname = (nc_prog.partition_id_tensor.name
             if nc_prog.partition_id_tensor else None)
    in_names, out_names, out_avals, zero_outs = [], [], [], []
    for alloc in nc_prog.m.functions[0].allocations:
        if not isinstance(alloc, _mb.MemoryLocationSet):
            continue
        name = alloc.memorylocations[0].name
        if alloc.kind == 'ExternalInput':
            if name != pname:
                in_names.append(name)
        elif alloc.kind == 'ExternalOutput':
            out_names.append(name)
            shape = tuple(alloc.tensor_shape)
            dtype = _mb.dt.np(alloc.dtype)
            out_avals.append(jax.core.ShapedArray(shape, dtype))
            zero_outs.append(np.zeros(shape, dtype))
    n_params = len(in_names)
    all_names = in_names + out_names + ([pname] if pname else [])

    def _body(*args):
        operands = list(args)
        if pname:
            operands.append(bass2jax.partition_id_tensor())
        outs = bass2jax._bass_exec_p.bind(
            *operands, out_avals=tuple(out_avals), in_names=tuple(all_names),
            out_names=tuple(out_names), lowering_input_output_aliases=(),
            sim_require_finite=True, sim_require_nnan=True, nc=nc_prog)
        return tuple(outs)

    devices = jax.devices()[:NC]
    mesh = Mesh(np.asarray(devices), ('core',))
    n_outs = len(out_names)
    sharded = jax.jit(
        shard_map(_body, mesh=mesh,
                  in_specs=(PartitionSpec('core'),) * (n_params + n_outs),
                  out_specs=(PartitionSpec('core'),) * n_outs,
                  check_rep=False),
        keep_unused=True)

    def run(in_maps):
        concat_in = [np.concatenate([np.asarray(m[nm]) for m in in_maps],
                                    axis=0) for nm in in_names]
        concat_zero = [np.zeros((NC * z.shape[0], *z.shape[1:]), z.dtype)
                       for z in zero_outs]
        out_arrs = sharded(*concat_in, *concat_zero)
        o = np.asarray(out_arrs[0]).reshape(NC, *out_avals[0].shape)
        return [o[c] for c in range(NC)]

    _RUN_CACHE[ck] = run
    return run


if __name__ == '__main__':
    sys.path.insert(0, '/root/problem')
    import jax
    import reference as R
    cpu = jax.devices('cpu')[0]
    with jax.default_device(cpu):
        inp = {k: np.asarray(v) for k, v in R.setup_inputs().items()}
        exp = np.asarray(R.reference(**R.setup_inputs()))
    got = kernel(**inp)
    rel = np.abs(got - exp) / (np.abs(exp) + 1e-9)
    print('expected:', exp[:8])
    print('got     :', got[:8])
    print('max rel err:', rel.max())
